# revision 1
# baseline (speedup 1.0000x reference)
"""Trainium2 Bass kernel for nn_Attention_Encoder (conv1x1 -> time-softmax attention -> relu-GRU).

Sharding: pure data parallelism. The folded batch*ltms segment axis (64*16=1024
segments) is split across 8 NeuronCores, 128 segments per core; weights are
replicated. Each core runs the pointwise conv, per-filter softmax attention
over time, and the 256-step GRU entirely on-chip; the gates_x matmuls are fused
into the recurrence's PSUM accumulation so nothing but the x shard and the
final h ever touch DRAM.

Layouts (per core, S=128 segments):
  phase A (per segment s, bf16 matmuls, fp32 PSUM):
    x_T   [C=128p, T=256] bf16   via transpose-DMA (xbar)
    conv_T[F(2ch), T] bf16 = relu(W_c^T x_T)  (ACT evac, per-chunk bias)
    conv_N[T(2ch), F] bf16 = relu(x_T^T W_c)  (DVE evac)
    scores[F(2ch), T] = conv_N^T A ; E = exp(scores) w/ fused row-sum
    x_att [F(2ch), T] = E * (1/sum) * conv_T -> global bf16 [128, 2, T, S]
  phase B (per step t, gate-major transposed layout):
    ps_r  [r(2ch), S]       = W_r^T x_att[t] + U_r^T h    (own PSUM bank ->
                              sigmoid_r fires after 4 U-matmuls)
    ps_zh [z(2ch) rh(2ch), S], ps_xh [xh(2ch), S]
    r,z = sigmoid(...); hh = relu(xh + r*rh)  (PSUM-direct DVE reads)
    h = hh + z*(h_prev - hh)
"""

import contextlib
import os
import sys

sys.path.insert(0, "/opt/trn_rl_repo")

import numpy as np
import ml_dtypes

import concourse.bass as bass
import concourse.tile as tile
from concourse import mybir
from concourse.bass_utils import run_bass_kernel_spmd

F32 = mybir.dt.float32
F32R = mybir.dt.float32r
BF16 = mybir.dt.bfloat16
AF = mybir.ActivationFunctionType
OP = mybir.AluOpType

B, LTMS, TTS, C_IN, FF, HH = 64, 16, 256, 128, 256, 256
NCORES = 8
S = (B * LTMS) // NCORES  # 128 segments per core
T = TTS                   # 256 timesteps

# bfpack column layout (bf16): conv_w | attn_w | gru_w | gru_u | identity
BP_CW = 0
BP_AW = BP_CW + FF              # 256
BP_WG = BP_AW + 2 * T           # 768
BP_WU = BP_WG + 2 * 3 * HH      # 2304
BP_ID = BP_WU + 2 * 3 * HH      # 3840
BP_W = BP_ID + 128              # 3968


def build(zero_bias: bool) -> bass.Bass:
    nc = bass.Bass("TRN2", target_bir_lowering=False)

    x_d = nc.dram_tensor("x_shard", [S, T, C_IN], BF16, kind="ExternalInput")
    bp_d = nc.dram_tensor("bfpack", [128, BP_W], BF16, kind="ExternalInput")
    if not zero_bias:
        cb_d = nc.dram_tensor("conv_b2", [128, 2], F32, kind="ExternalInput")
        ab_d = nc.dram_tensor("attn_b", [1, T], BF16, kind="ExternalInput")
        gb_d = nc.dram_tensor("gbias", [S, 8], F32, kind="ExternalInput")
    out_d = nc.dram_tensor("h_out", [S, HH], F32, kind="ExternalOutput")

    with tile.TileContext(nc, trace_sim=bool(os.environ.get("KTRACE"))) as tc:
        with contextlib.ExitStack() as ctx:
            singles = ctx.enter_context(tc.tile_pool(name="singles", bufs=1))

            bp_sb = singles.tile([128, BP_W], BF16)
            nc.sync.dma_start(bp_sb, bp_d[:])

            cw_sb = bp_sb[:, BP_CW:BP_CW + FF]
            aw_sb = bp_sb[:, BP_AW:BP_AW + 2 * T].rearrange(
                "p (k n) -> p k n", k=2)
            wg_sb = bp_sb[:, BP_WG:BP_WG + 1536].rearrange(
                "p (k n) -> p k n", k=2)
            wu_sb = bp_sb[:, BP_WU:BP_WU + 1536].rearrange(
                "p (k n) -> p k n", k=2)
            ident_bf = bp_sb[:, BP_ID:BP_ID + 128]

            # global x_att store: [F%128, F-chunk, T, S] bf16
            xatt = singles.tile([128, 2, T, S], BF16)


            if not zero_bias:
                cb_sb = singles.tile([128, 2], F32)
                nc.sync.dma_start(cb_sb, cb_d[:])
                ab_row = singles.tile([1, T], BF16)
                nc.sync.dma_start(ab_row, ab_d[:])
                ones_col = singles.tile([1, 128], BF16)
                nc.vector.memset(ones_col, 1.0)
                gb_sb = singles.tile([128, 8], F32)
                nc.sync.dma_start(gb_sb, gb_d[:])

            # ---------------- phase A ----------------
            apool = ctx.enter_context(tc.tile_pool(name="apool", bufs=3))
            with contextlib.ExitStack() as actx:
                apsum = actx.enter_context(
                    tc.tile_pool(name="apsum", bufs=2, space="PSUM"))

                # PE warmup: consume the weight-pack DMA on PE so its queue
                # sem enters PE's vector clock (keeps matmul waits small)
                ps_w1 = apsum.tile([128, 128], BF16, tag="ps_ct", bufs=1)
                nc.tensor.transpose(ps_w1, ident_bf, ident_bf)

                for s2 in range(S // 2):
                    s = 2 * s2
                    x_t = apool.tile([128, 2, T], BF16, tag="x_t", bufs=4)
                    nc.sync.dma_start_transpose(x_t[:, 0, :], x_d[s])
                    nc.sync.dma_start_transpose(x_t[:, 1, :], x_d[s + 1])

                    # conv_T = relu(W_c^T @ x_T): [F(2ch), seg, T]
                    ps_ct = apsum.tile([128, 2, 2, T], F32, tag="ps_ct", bufs=1)
                    for m in range(2):
                        nc.tensor.matmul(
                            ps_ct[:, m, :, :], cw_sb[:, bass.ts(m, 128)],
                            x_t, start=True, stop=True)
                    conv_t = apool.tile([128, 2, 2, T], BF16, tag="conv_t")
                    # balance: chunk 0 relu on ACT, chunk 1 on DVE
                    nc.scalar.activation(
                        conv_t[:, 0, :, :], ps_ct[:, 0, :, :], AF.Relu,
                        bias=0.0 if zero_bias else cb_sb[:, 0:1])
                    if zero_bias:
                        nc.vector.tensor_scalar_max(
                            conv_t[:, 1, :, :], ps_ct[:, 1, :, :], 0.0)
                    else:
                        nc.vector.tensor_scalar(
                            conv_t[:, 1, :, :], ps_ct[:, 1, :, :],
                            cb_sb[:, 1:2], 0.0, OP.add, OP.max)

                    # conv_N = relu(x_T^T @ W_c): [seg, T-ch, F]
                    ps_cn = apsum.tile([128, 2, 2, FF], F32, tag="ps_cs", bufs=3)
                    for seg in range(2):
                        for k in range(2):
                            nc.tensor.matmul(
                                ps_cn[:, seg, k, :],
                                x_t[:, seg, bass.ts(k, 128)],
                                cw_sb, start=True, stop=True)
                    conv_n = apool.tile([128, 2, 2, FF], BF16, tag="conv_n")
                    for seg in range(2):
                        nc.vector.tensor_scalar_max(
                            conv_n[:, seg, :, :], ps_cn[:, seg, :, :], 0.0)

                    # scores = conv_N^T @ A (+ b): [seg, F-ch, T]
                    ps_s = apsum.tile([128, 2, 2, T], F32, tag="ps_cs", bufs=3)
                    for seg in range(2):
                        for m in range(2):
                            for k in range(2):
                                nc.tensor.matmul(
                                    ps_s[:, seg, m, :],
                                    conv_n[:, seg, k, bass.ts(m, 128)],
                                    aw_sb[:, k, :],
                                    start=(k == 0),
                                    stop=(k == 1) and zero_bias)
                            if not zero_bias:
                                nc.tensor.matmul(
                                    ps_s[:, seg, m, :], ones_col, ab_row,
                                    start=False, stop=True)

                    ee = apool.tile([128, 2, 2, T], BF16, tag="ee")
                    esum = apool.tile([128, 2, 2, 1], F32, tag="esum")
                    for seg in range(2):
                        for m in range(2):
                            nc.scalar.activation(
                                ee[:, seg, m, :], ps_s[:, seg, m, :], AF.Exp,
                                accum_out=esum[:, seg, m, :])
                    rinv = apool.tile([128, 2, 2, 1], F32, tag="rinv")
                    for seg in range(2):
                        for m in range(2):
                            nc.vector.reciprocal(
                                rinv[:, seg, m, :], esum[:, seg, m, :])

                    # x_att = E * rinv * conv_T -> global bf16 columns s, s+1
                    for seg in range(2):
                        for m in range(2):
                            nc.vector.scalar_tensor_tensor(
                                out=xatt[:, m, :, s + seg],
                                in0=ee[:, seg, m, :],
                                scalar=rinv[:, seg, m, :],
                                in1=conv_t[:, m, seg, :],
                                op0=OP.mult,
                                op1=OP.mult)

            # ---------------- phase B: GRU over T steps ----------------
            # gate column order in W/U: z=[0,256) m0,1 ; r=[256,512) m2,3 ;
            # h=[512,768) m4,5
            with contextlib.ExitStack() as bctx:
                hpool = bctx.enter_context(tc.tile_pool(name="hpool", bufs=2))
                gpool = bctx.enter_context(tc.tile_pool(name="gpool", bufs=3))
                bpsum = bctx.enter_context(
                    tc.tile_pool(name="bpsum", bufs=1, space="PSUM"))

                h_prev = None
                for t in range(T):
                    ps_r = bpsum.tile([128, 2, S], F32, tag="ps_r")
                    ps_z = bpsum.tile([128, 2, S], F32, tag="ps_z")
                    ps_rx = bpsum.tile([128, 4, S], F32, tag="ps_rx")

                    # x-part matmuls (independent of h)
                    for j, m in enumerate((2, 3)):      # r gates
                        for k in range(2):
                            nc.tensor.matmul(
                                ps_r[:, j, :], wg_sb[:, k, bass.ts(m, 128)],
                                xatt[:, k, t, :],
                                start=(k == 0), stop=(k == 1) and (t == 0))
                    for j, m in enumerate((0, 1)):      # z gates
                        for k in range(2):
                            nc.tensor.matmul(
                                ps_z[:, j, :], wg_sb[:, k, bass.ts(m, 128)],
                                xatt[:, k, t, :],
                                start=(k == 0), stop=(k == 1) and (t == 0))
                    for j, m in enumerate((4, 5)):      # h gate (xh)
                        for k in range(2):
                            nc.tensor.matmul(
                                ps_rx[:, 2 + j, :],
                                wg_sb[:, k, bass.ts(m, 128)],
                                xatt[:, k, t, :],
                                start=(k == 0), stop=(k == 1))
                    if t > 0:
                        # U-part: r first (gates sigmoid_r), then rh, then z
                        for j, m in enumerate((2, 3)):
                            for k in range(2):
                                nc.tensor.matmul(
                                    ps_r[:, j, :],
                                    wu_sb[:, k, bass.ts(m, 128)],
                                    h_prev[:, k, :],
                                    start=False, stop=(k == 1))
                        for j, m in enumerate((4, 5)):  # rh -> ps_rx[0:2]
                            for k in range(2):
                                nc.tensor.matmul(
                                    ps_rx[:, j, :],
                                    wu_sb[:, k, bass.ts(m, 128)],
                                    h_prev[:, k, :],
                                    start=(k == 0), stop=(k == 1))
                        for j, m in enumerate((0, 1)):
                            for k in range(2):
                                nc.tensor.matmul(
                                    ps_z[:, j, :],
                                    wu_sb[:, k, bass.ts(m, 128)],
                                    h_prev[:, k, :],
                                    start=False, stop=(k == 1))

                    xh_sb = gpool.tile([128, 2, S], BF16, tag="xh_sb")
                    if zero_bias:
                        nc.vector.tensor_copy(xh_sb, ps_rx[:, 2:4, :])
                    else:
                        for j in range(2):
                            nc.vector.tensor_scalar_add(
                                xh_sb[:, j, :], ps_rx[:, 2 + j, :],
                                gb_sb[:, 6 + j : 7 + j])

                    r = gpool.tile([128, 2, S], BF16, tag="rt")
                    z = gpool.tile([128, 2, S], BF16, tag="zt")
                    if zero_bias:
                        nc.scalar.activation(r, ps_r, AF.Sigmoid)
                        nc.scalar.activation(z, ps_z, AF.Sigmoid)
                    else:
                        for j, m in enumerate((2, 3)):
                            nc.scalar.activation(
                                r[:, j, :], ps_r[:, j, :], AF.Sigmoid,
                                bias=gb_sb[:, m : m + 1])
                        for j, m in enumerate((0, 1)):
                            nc.scalar.activation(
                                z[:, j, :], ps_z[:, j, :], AF.Sigmoid,
                                bias=gb_sb[:, m : m + 1])

                    h_new = hpool.tile([128, 2, S], BF16, tag="h")
                    hht = gpool.tile([128, 2, S], BF16, tag="hht")
                    tt = gpool.tile([128, 2, S], BF16, tag="tt")
                    if t > 0:
                        # hh = relu(xh + r*rh); rh straight from PSUM
                        if zero_bias:
                            nc.vector.tensor_mul(tt, r, ps_rx[:, 0:2, :])
                        else:
                            for j in range(2):
                                nc.vector.scalar_tensor_tensor(
                                    out=tt[:, j, :], in0=ps_rx[:, j, :],
                                    scalar=gb_sb[:, 4 + j : 5 + j],
                                    in1=r[:, j, :], op0=OP.add, op1=OP.mult)
                        nc.vector.tensor_add(tt, tt, xh_sb)
                        nc.vector.tensor_scalar_max(hht, tt, 0.0)
                        # h = hht + z*(h_prev - hht)
                        dd = gpool.tile([128, 2, S], BF16, tag="dd")
                        nc.vector.tensor_sub(dd, h_prev, hht)
                        nc.vector.tensor_mul(dd, z, dd)
                        nc.vector.tensor_add(h_new, hht, dd)
                    else:
                        if zero_bias:
                            nc.vector.tensor_scalar_max(hht, xh_sb, 0.0)
                        else:
                            for j in range(2):
                                nc.vector.tensor_scalar_mul(
                                    tt[:, j, :], r[:, j, :],
                                    gb_sb[:, 4 + j : 5 + j])
                            nc.vector.tensor_add(tt, tt, xh_sb)
                            nc.vector.tensor_scalar_max(hht, tt, 0.0)
                        wt = gpool.tile([128, 2, S], BF16, tag="tt")
                        nc.vector.tensor_scalar(wt, z, -1.0, 1.0, OP.mult,
                                                OP.add)
                        nc.vector.tensor_mul(h_new, wt, hht)

                    h_prev = h_new

                # output: transpose h back to [S, H] and store fp32
                ps_o = bpsum.tile([128, 2, S], BF16, tag="ps_r")
                for c in range(2):
                    nc.tensor.transpose(ps_o[:, c, :], h_prev[:, c, :],
                                        ident_bf)
                out_sb = gpool.tile([128, 2, 128], F32, tag="out_sb")
                nc.vector.tensor_copy(out_sb, ps_o)
                nc.sync.dma_start(
                    out_d.rearrange("s (c p) -> s c p", c=2), out_sb)

    _split_multi_waits(nc)
    return nc


def _split_multi_waits(nc: bass.Bass):
    """This walrus encodes at most ONE semaphore wait per ISA instruction.
    Tile's sem assignment can attach several; hoist the excess onto
    preceding same-engine NoOp carriers (the sequencer executes them in
    order, so semantics are identical)."""
    fn = nc.m.functions[0]
    for blk in fn.blocks:
        insts = list(blk.instructions)
        out = []
        changed = False
        for inst in insts:
            si = inst.sync_info
            waits = list(si.on_wait) if si is not None else []
            if len(waits) > 1:
                changed = True
                for w in waits[:-1]:
                    out.append(mybir.InstNoOp(
                        name=f"I-wsplit-{nc.next_id()}",
                        engine=inst.engine,
                        ins=[], outs=[],
                        sync_info=mybir.SyncInfo(on_wait=[w], on_update=[]),
                    ))
                inst.sync_info = mybir.SyncInfo(
                    on_wait=[waits[-1]], on_update=list(si.on_update))
            out.append(inst)
        if changed:
            blk.instructions = out


_CACHE = {}


def _get_nc(zero_bias: bool) -> bass.Bass:
    key = zero_bias
    if key not in _CACHE:
        _CACHE[key] = build(zero_bias)
    return _CACHE[key]


def _pack_weights(conv_w, attn_w, gru_w, gru_u):
    bf = ml_dtypes.bfloat16
    cw = (conv_w[0] if conv_w.ndim == 3 else conv_w).astype(bf)  # [128, 256]
    aw = attn_w.astype(bf).reshape(2, 128, T).transpose(1, 0, 2).reshape(
        128, 2 * T)
    wg = gru_w.astype(bf).reshape(2, 128, 768).transpose(1, 0, 2).reshape(
        128, 1536)
    wu = gru_u.astype(bf).reshape(2, 128, 768).transpose(1, 0, 2).reshape(
        128, 1536)
    ident = np.eye(128, dtype=np.float32).astype(bf)
    return np.ascontiguousarray(
        np.concatenate([cw, aw, wg, wu, ident], axis=1), bf)


def kernel(x, conv_w, conv_b, attn_w, attn_b, gru_w, gru_u, gru_b):
    x = np.asarray(x, dtype=np.float32)
    conv_w = np.asarray(conv_w, dtype=np.float32)
    conv_b = np.asarray(conv_b, dtype=np.float32)
    attn_w = np.asarray(attn_w, dtype=np.float32)
    attn_b = np.asarray(attn_b, dtype=np.float32)
    gru_w = np.asarray(gru_w, dtype=np.float32)
    gru_u = np.asarray(gru_u, dtype=np.float32)
    gru_b = np.asarray(gru_b, dtype=np.float32)

    zero_bias = (
        not conv_b.any() and not attn_b.any() and not gru_b.any())

    nc = _get_nc(zero_bias)

    xs_bf = x.reshape(B * LTMS, T, C_IN).astype(ml_dtypes.bfloat16)
    bfpack = _pack_weights(conv_w, attn_w, gru_w, gru_u)

    in_maps = []
    for c in range(NCORES):
        m = {
            "x_shard": np.ascontiguousarray(xs_bf[c * S : (c + 1) * S]),
            "bfpack": bfpack,
        }
        if not zero_bias:
            bi, br = gru_b[0], gru_b[1]
            comb = bi + br
            gb = np.zeros((128, 8), np.float32)
            for ch in range(4):
                gb[:, ch] = comb[ch * 128 : (ch + 1) * 128]
            gb[:, 4] = br[512:640]
            gb[:, 5] = br[640:768]
            gb[:, 6] = bi[512:640]
            gb[:, 7] = bi[640:768]
            m["conv_b2"] = np.ascontiguousarray(
                conv_b.reshape(2, 128).T, np.float32)
            m["attn_b"] = attn_b.reshape(1, T).astype(ml_dtypes.bfloat16)
            m["gbias"] = gb
        in_maps.append(m)

    res = run_bass_kernel_spmd(nc, in_maps, core_ids=list(range(NCORES)))
    outs = [res.results[c]["h_out"] for c in range(NCORES)]
    h = np.concatenate(outs, axis=0)  # [1024, 256]
    return h.reshape(B, LTMS, HH).astype(np.float32)


if __name__ == "__main__":
    nc = _get_nc(True)
    print("built ok")



# revision 2
# speedup vs baseline: 1.1421x; 1.1421x over previous
"""Trainium2 Bass kernel for nn_Attention_Encoder (conv1x1 -> time-softmax attention -> relu-GRU).

Sharding: pure data parallelism. 1024 segments split across 8 cores (S=128
per core); weights replicated. v2 redesign vs baseline:

phase A (per segment pair):
  x_T [C,2,T] via transpose-DMA; conv_T = relu(Wc^T x_T) (DVE TSP evac);
  conv_N obtained by PE transposes of the relu'd conv_T (bf16 PSUM
  pass-through, evacuated by a 2x-rate TensorCopy); scores -> exp (ACT,
  fused row-sum); x_att = E*conv on Pool (TT), then *rinv on DVE (4x TSP),
  stored [128, k, S, T] so writes are packed.

phase B: two software-pipelined chains of SC=64 segments (B half a step
  behind A) hide the recurrence latency. Per chain-step: one PSUM tile
  [128, 8, SC] holds z,r,rh,xh (concurrent per-bank accumulation groups,
  sim check skipped -- hardware zeroes only written bytes); ACT evacuates
  xh and runs one combined sigmoid over [z;r]; DVE: t1=r*rh(PSUM),
  q=t1+xh, hh=relu(q) (4x TSP); Pool: d=h-hh, e=z*d, h'=hh+e.
  GRU biases (generic path) are added via rank-1 matmuls into PSUM.
"""

import contextlib
import os
import sys

sys.path.insert(0, "/opt/trn_rl_repo")

import numpy as np
import ml_dtypes

import concourse.bass as bass
import concourse.tile as tile
from concourse import mybir
from concourse.bass_utils import run_bass_kernel_spmd

F32 = mybir.dt.float32
BF16 = mybir.dt.bfloat16
AF = mybir.ActivationFunctionType
OP = mybir.AluOpType

B, LTMS, TTS, C_IN, FF, HH = 64, 16, 256, 128, 256, 256
NCORES = 8
S = (B * LTMS) // NCORES  # 128 segments per core
T = TTS                   # 256 timesteps
SC = S // 2               # 64 segments per chain

# bfpack column layout (bf16): conv_w | attn_w | gru_w | gru_u | identity
BP_CW = 0
BP_AW = BP_CW + FF              # 256
BP_WG = BP_AW + 2 * T           # 768
BP_WU = BP_WG + 2 * 3 * HH      # 2304
BP_ID = BP_WU + 2 * 3 * HH      # 3840
BP_W = BP_ID + 128              # 3968


def build(zero_bias: bool) -> bass.Bass:
    nc = bass.Bass("TRN2", target_bir_lowering=False)

    x_d = nc.dram_tensor("x_shard", [S, T, C_IN], BF16, kind="ExternalInput")
    bp_d = nc.dram_tensor("bfpack", [128, BP_W], BF16, kind="ExternalInput")
    if not zero_bias:
        cb_d = nc.dram_tensor("conv_b2", [128, 2], F32, kind="ExternalInput")
        ab_d = nc.dram_tensor("attn_b", [1, T], BF16, kind="ExternalInput")
        # gru bias rows for rank-1 PSUM adds: [1, 8*128] bf16
        # order: z0 z1 r0 r1 (bi+br) | rh0 rh1 (br_h) | xh0 xh1 (bi_h)
        gb_d = nc.dram_tensor("gbias_row", [1, 8 * 128], BF16,
                              kind="ExternalInput")
    out_d = nc.dram_tensor("h_out", [S, HH], F32, kind="ExternalOutput")

    with tile.TileContext(nc, trace_sim=bool(os.environ.get("KTRACE"))) as tc:
        with contextlib.ExitStack() as ctx:
            singles = ctx.enter_context(tc.tile_pool(name="singles", bufs=1))

            bp_sb = singles.tile([128, BP_W], BF16)
            nc.sync.dma_start(bp_sb, bp_d[:])

            cw_sb = bp_sb[:, BP_CW:BP_CW + FF]
            aw_sb = bp_sb[:, BP_AW:BP_AW + 2 * T].rearrange(
                "p (k n) -> p k n", k=2)
            wg_sb = bp_sb[:, BP_WG:BP_WG + 1536].rearrange(
                "p (k n) -> p k n", k=2)
            wu_sb = bp_sb[:, BP_WU:BP_WU + 1536].rearrange(
                "p (k n) -> p k n", k=2)
            ident_bf = bp_sb[:, BP_ID:BP_ID + 128]

            # global x_att store: [F%128, F-chunk, S, T] bf16 (T packed)
            xatt = singles.tile([128, 2, S, T], BF16)

            if not zero_bias:
                cb_sb = singles.tile([128, 2], F32)
                nc.sync.dma_start(cb_sb, cb_d[:])
                ab_row = singles.tile([1, T], BF16)
                nc.sync.dma_start(ab_row, ab_d[:])
                gb_row = singles.tile([1, 8 * 128], BF16)
                nc.sync.dma_start(gb_row, gb_d[:])
                ones_col = singles.tile([1, 128], BF16)
                nc.vector.memset(ones_col, 1.0)
                ones_sc = ones_col[:, :SC]

            # ---------------- phase A ----------------
            apool = ctx.enter_context(tc.tile_pool(name="apool", bufs=3))
            with contextlib.ExitStack() as actx:
                apsum = actx.enter_context(
                    tc.tile_pool(name="apsum", bufs=1, space="PSUM"))

                # PE warmup: consume the weight-pack DMA on PE early
                # (borrows the ps_cn tag so phase A stays within 8 PSUM banks)
                ps_w1 = apsum.tile([128, 128], BF16, tag="ps_cn", bufs=2)
                nc.tensor.transpose(ps_w1, ident_bf, ident_bf)

                for s2 in range(S // 2):
                    s = 2 * s2
                    x_t = apool.tile([128, 2, T], BF16, tag="x_t", bufs=4)
                    nc.sync.dma_start_transpose(x_t[:, 0, :], x_d[s])
                    nc.sync.dma_start_transpose(x_t[:, 1, :], x_d[s + 1])

                    # conv_T = relu(W_c^T @ x_T): [F(2ch), seg, T]
                    ps_ct = apsum.tile([128, 2, 2, T], F32, tag="ps_ct",
                                       bufs=1)
                    for m in range(2):
                        nc.tensor.matmul(
                            ps_ct[:, m, :, :], cw_sb[:, bass.ts(m, 128)],
                            x_t, start=True, stop=True)
                    conv_t = apool.tile([128, 2, 2, T], BF16, tag="conv_t")
                    # evac+relu: chunk 0 on DVE; chunk 1 split DVE/ACT to
                    # balance the two engines across pairs
                    nc.vector.tensor_scalar(
                        conv_t[:, 0, :, :], ps_ct[:, 0, :, :],
                        0.0 if zero_bias else cb_sb[:, 0:1], 0.0,
                        OP.add, OP.max)
                    if False:
                        nc.scalar.activation(
                            conv_t[:, 1, :, :], ps_ct[:, 1, :, :], AF.Relu,
                            bias=0.0 if zero_bias else cb_sb[:, 1:2])
                    else:
                        nc.vector.tensor_scalar(
                            conv_t[:, 1, :, :], ps_ct[:, 1, :, :],
                            0.0 if zero_bias else cb_sb[:, 1:2], 0.0,
                            OP.add, OP.max)

                    # conv_N via PE transposes of relu'd conv_T (bf16 psum)
                    ps_cn = apsum.tile([128, 2, 2, FF], BF16, tag="ps_cn",
                                       bufs=2)
                    for seg in range(2):
                        for tch in range(2):
                            for m in range(2):
                                nc.tensor.transpose(
                                    ps_cn[:, seg, tch, bass.ts(m, 128)],
                                    conv_t[:, m, seg, bass.ts(tch, 128)],
                                    ident_bf)
                    conv_n = apool.tile([128, 2, 2, FF], BF16, tag="conv_n")
                    for seg in range(2):
                        nc.vector.tensor_copy(
                            conv_n[:, seg, :, :], ps_cn[:, seg, :, :])

                    # scores = conv_N^T @ A (+ b): [seg, F-ch, T]
                    ps_s = apsum.tile([128, 2, 2, T], F32, tag="ps_s", bufs=2)
                    for seg in range(2):
                        for m in range(2):
                            for k in range(2):
                                nc.tensor.matmul(
                                    ps_s[:, seg, m, :],
                                    conv_n[:, seg, k, bass.ts(m, 128)],
                                    aw_sb[:, k, :],
                                    start=(k == 0),
                                    stop=(k == 1) and zero_bias)
                            if not zero_bias:
                                nc.tensor.matmul(
                                    ps_s[:, seg, m, :], ones_col, ab_row,
                                    start=False, stop=True)

                    ee = apool.tile([128, 2, 2, T], BF16, tag="ee")
                    esum = apool.tile([128, 4], F32, tag="esum")
                    es4 = esum.rearrange("p (a b) -> p a b", a=2)
                    for seg in range(2):
                        for m in range(2):
                            nc.scalar.activation(
                                ee[:, seg, m, :], ps_s[:, seg, m, :], AF.Exp,
                                accum_out=es4[:, seg, m:m + 1])
                    rinv = apool.tile([128, 4], F32, tag="rinv")
                    nc.vector.reciprocal(rinv, esum)
                    ri4 = rinv.rearrange("p (a b) -> p a b", a=2)

                    # x_att[:, m, s+seg, :] = E * rinv * conv_T  (packed T)
                    # split: ec = E*conv on Pool (TT), then *rinv on DVE (4x)
                    ec = apool.tile([128, 2, 2, T], BF16, tag="ec")
                    for seg in range(2):
                        for m in range(2):
                            nc.gpsimd.tensor_mul(
                                ec[:, seg, m, :], ee[:, seg, m, :],
                                conv_t[:, m, seg, :])
                    for seg in range(2):
                        for m in range(2):
                            nc.vector.tensor_scalar_mul(
                                xatt[:, m, s + seg, :], ec[:, seg, m, :],
                                ri4[:, seg, m:m + 1])

            # ---------------- phase B: GRU over T steps, 2 chains ----------
            # gate columns in W/U: z=[0,256) m0,1 ; r=[256,512) m2,3 ;
            # h=[512,768) m4,5
            # psum tile layout [128, 8, SC]: z0 z1 r0 r1 | rh0 rh1 | xh0 xh1
            with contextlib.ExitStack() as bctx:
                hpool = bctx.enter_context(tc.tile_pool(name="hpool", bufs=2))
                gpool = bctx.enter_context(tc.tile_pool(name="gpool", bufs=3))
                bpsum = bctx.enter_context(
                    tc.tile_pool(name="bpsum", bufs=1, space="PSUM"))

                h_prev = [None, None]
                pend = [None, None]  # (t, ps, rz, xh_sb) awaiting elementwise

                def emit_pe_act(c, t):
                    """Matmuls + sigmoid + xh evac for (chain c, step t)."""
                    cb = c * SC
                    ps = bpsum.tile([128, 8, SC], F32, tag=f"ps{c}", bufs=3,
                                    name=f"ps{c}")
                    hp = h_prev[c]

                    # x-part matmuls (independent of h)
                    zr_stop = (t == 0) and zero_bias
                    for j, m in enumerate((0, 1)):      # z gates
                        for k in range(2):
                            nc.tensor.matmul(
                                ps[:, j, :], wg_sb[:, k, bass.ts(m, 128)],
                                xatt[:, k, cb:cb + SC, t],
                                start=(k == 0), stop=(k == 1) and zr_stop,
                                skip_group_check=True)
                    for j, m in enumerate((2, 3)):      # r gates
                        for k in range(2):
                            nc.tensor.matmul(
                                ps[:, 2 + j, :],
                                wg_sb[:, k, bass.ts(m, 128)],
                                xatt[:, k, cb:cb + SC, t],
                                start=(k == 0), stop=(k == 1) and zr_stop,
                                skip_group_check=True)
                    for j, m in enumerate((4, 5)):      # h gate (xh)
                        for k in range(2):
                            nc.tensor.matmul(
                                ps[:, 6 + j, :],
                                wg_sb[:, k, bass.ts(m, 128)],
                                xatt[:, k, cb:cb + SC, t],
                                start=(k == 0),
                                stop=(k == 1) and zero_bias,
                                skip_group_check=True)

                    if not zero_bias:
                        # rank-1 bias adds; z0..r1 into [0:4],
                        # xh into [6:8], rh (br_h) into [4:6]
                        for j in range(4):
                            nc.tensor.matmul(
                                ps[:, j, :], gb_row[:, bass.ts(j, 128)],
                                ones_sc, start=False, stop=(t == 0),
                                skip_group_check=True)
                        for j in range(2):
                            nc.tensor.matmul(
                                ps[:, 6 + j, :],
                                gb_row[:, bass.ts(6 + j, 128)],
                                ones_sc, start=False, stop=True,
                                skip_group_check=True)
                        for j in range(2):
                            nc.tensor.matmul(
                                ps[:, 4 + j, :],
                                gb_row[:, bass.ts(4 + j, 128)],
                                ones_sc, start=True, stop=(t == 0),
                                skip_group_check=True)

                    # ACT evacuates xh early (depends only on Wx)
                    xh_sb = gpool.tile([128, 2, SC], BF16, tag=f"xh{c}",
                                       bufs=2, name=f"xh{c}")
                    nc.scalar.copy(xh_sb, ps[:, 6:8, :])

                    rz = gpool.tile([128, 4, SC], BF16, tag=f"rz{c}",
                                    bufs=2, name=f"rz{c}")
                    if t > 0:
                        # U-part: r,z first (gate the sigmoid), then rh
                        for j, m in enumerate((2, 3)):
                            for k in range(2):
                                nc.tensor.matmul(
                                    ps[:, 2 + j, :],
                                    wu_sb[:, k, bass.ts(m, 128)],
                                    hp[:, k, :],
                                    start=False, stop=(k == 1),
                                    skip_group_check=True)
                        for j, m in enumerate((0, 1)):
                            for k in range(2):
                                nc.tensor.matmul(
                                    ps[:, j, :],
                                    wu_sb[:, k, bass.ts(m, 128)],
                                    hp[:, k, :],
                                    start=False, stop=(k == 1),
                                    skip_group_check=True)
                        for j, m in enumerate((4, 5)):  # rh
                            for k in range(2):
                                nc.tensor.matmul(
                                    ps[:, 4 + j, :],
                                    wu_sb[:, k, bass.ts(m, 128)],
                                    hp[:, k, :],
                                    start=(k == 0) and zero_bias,
                                    stop=(k == 1),
                                    skip_group_check=True)
                    # sigmoid over [z;r] in one ACT op
                    nc.scalar.activation(rz, ps[:, 0:4, :], AF.Sigmoid)
                    pend[c] = (t, ps, rz, xh_sb)

                def emit_dve(c):
                    """Elementwise chain for the pending (chain c) step.
                    The d/e blend ops run on Pool to cut DVE occupancy."""
                    t, ps, rz, xh_sb = pend[c]
                    hp = h_prev[c]
                    h_new = hpool.tile([128, 2, SC], BF16, tag=f"h{c}",
                                       name=f"h{c}")
                    hh = gpool.tile([128, 2, SC], BF16, tag=f"hh{c}",
                                    bufs=2, name=f"hh{c}")
                    d = gpool.tile([128, 2, SC], BF16, tag=f"d{c}",
                                   bufs=2, name=f"d{c}")
                    e = gpool.tile([128, 2, SC], BF16, tag=f"e{c}",
                                   bufs=2, name=f"e{c}")
                    have_rh = (t > 0) or not zero_bias
                    if have_rh:
                        t1 = gpool.tile([128, 2, SC], BF16, tag=f"t1{c}",
                                        bufs=2, name=f"t1{c}")
                        q = gpool.tile([128, 2, SC], BF16, tag=f"q{c}",
                                       bufs=2, name=f"q{c}")
                        nc.vector.tensor_mul(t1, rz[:, 2:4, :], ps[:, 4:6, :])
                        nc.vector.tensor_add(q, t1, xh_sb)
                    else:
                        q = xh_sb
                    # hh = relu(q) (TSP, 4x); then the z-blend
                    nc.vector.tensor_scalar_max(hh, q, 0.0)
                    if t > 0:
                        nc.gpsimd.tensor_sub(d, hp, hh)        # d = h - hh
                        nc.gpsimd.tensor_mul(e, rz[:, 0:2, :], d)
                        nc.gpsimd.tensor_add(h_new, hh, e)     # h' = hh+z(h-hh)
                    else:
                        nc.gpsimd.tensor_mul(e, rz[:, 0:2, :], hh)
                        nc.gpsimd.tensor_sub(h_new, hh, e)     # (1-z)*hh
                    h_prev[c] = h_new

                # software-pipelined: chain B runs half a step behind A.
                # Absolute-time pins phase-lock the two chains: a pin that's
                # already past is a no-op, so transient overruns self-correct.
                pin_base = float(os.environ.get("PIN_BASE", "228000"))
                pin_p = float(os.environ.get("PIN_P", "2300"))
                for t in range(T):
                    tp = pin_base + t * pin_p
                    with tc.tile_wait_until(tp / 1e6, enable=pin_p > 0):
                        emit_pe_act(0, t)
                    if t > 0:
                        with tc.tile_wait_until(tp / 1e6,
                                                enable=pin_p > 0):
                            emit_dve(1)
                    with tc.tile_wait_until((tp + 0.5 * pin_p) / 1e6,
                                            enable=pin_p > 0):
                        emit_pe_act(1, t)
                    with tc.tile_wait_until((tp + 0.46 * pin_p) / 1e6,
                                            enable=pin_p > 0):
                        emit_dve(0)
                emit_dve(1)

                # output: transpose h back to [S, H] and store fp32
                ps_o = bpsum.tile([64, 2, 2, 128], BF16, tag="ps_o", bufs=1)
                for c in range(2):
                    for ch in range(2):
                        nc.tensor.transpose(
                            ps_o[:, c, ch, :], h_prev[c][:, ch, :], ident_bf)
                out_sb = gpool.tile([64, 2, 2, 128], F32, tag="out_sb")
                nc.vector.tensor_copy(out_sb, ps_o)
                for c in range(2):
                    nc.sync.dma_start(
                        out_d[c * SC:(c + 1) * SC].rearrange(
                            "s (ch p) -> s ch p", ch=2), out_sb[:, c])

    _split_multi_waits(nc)
    return nc


def _split_multi_waits(nc: bass.Bass):
    """Encode at most ONE semaphore wait per ISA instruction: hoist extras
    onto preceding same-engine NoOp carriers."""
    fn = nc.m.functions[0]
    for blk in fn.blocks:
        insts = list(blk.instructions)
        out = []
        changed = False
        for inst in insts:
            si = inst.sync_info
            waits = list(si.on_wait) if si is not None else []
            if len(waits) > 1:
                changed = True
                for w in waits[:-1]:
                    out.append(mybir.InstNoOp(
                        name=f"I-wsplit-{nc.next_id()}",
                        engine=inst.engine,
                        ins=[], outs=[],
                        sync_info=mybir.SyncInfo(on_wait=[w], on_update=[]),
                    ))
                inst.sync_info = mybir.SyncInfo(
                    on_wait=[waits[-1]], on_update=list(si.on_update))
            out.append(inst)
        if changed:
            blk.instructions = out


_CACHE = {}


def _get_nc(zero_bias: bool) -> bass.Bass:
    if zero_bias not in _CACHE:
        _CACHE[zero_bias] = build(zero_bias)
    return _CACHE[zero_bias]


def _pack_weights(conv_w, attn_w, gru_w, gru_u):
    bf = ml_dtypes.bfloat16
    cw = (conv_w[0] if conv_w.ndim == 3 else conv_w).astype(bf)  # [128, 256]
    aw = attn_w.astype(bf).reshape(2, 128, T).transpose(1, 0, 2).reshape(
        128, 2 * T)
    wg = gru_w.astype(bf).reshape(2, 128, 768).transpose(1, 0, 2).reshape(
        128, 1536)
    wu = gru_u.astype(bf).reshape(2, 128, 768).transpose(1, 0, 2).reshape(
        128, 1536)
    ident = np.eye(128, dtype=np.float32).astype(bf)
    return np.ascontiguousarray(
        np.concatenate([cw, aw, wg, wu, ident], axis=1), bf)


def kernel(x, conv_w, conv_b, attn_w, attn_b, gru_w, gru_u, gru_b):
    x = np.asarray(x, dtype=np.float32)
    conv_w = np.asarray(conv_w, dtype=np.float32)
    conv_b = np.asarray(conv_b, dtype=np.float32)
    attn_w = np.asarray(attn_w, dtype=np.float32)
    attn_b = np.asarray(attn_b, dtype=np.float32)
    gru_w = np.asarray(gru_w, dtype=np.float32)
    gru_u = np.asarray(gru_u, dtype=np.float32)
    gru_b = np.asarray(gru_b, dtype=np.float32)

    zero_bias = (
        not conv_b.any() and not attn_b.any() and not gru_b.any())

    nc = _get_nc(zero_bias)

    xs_bf = x.reshape(B * LTMS, T, C_IN).astype(ml_dtypes.bfloat16)
    bfpack = _pack_weights(conv_w, attn_w, gru_w, gru_u)

    in_maps = []
    for c in range(NCORES):
        m = {
            "x_shard": np.ascontiguousarray(xs_bf[c * S: (c + 1) * S]),
            "bfpack": bfpack,
        }
        if not zero_bias:
            bi, br = gru_b[0], gru_b[1]
            comb = bi + br
            gbr = np.zeros((1, 8 * 128), np.float32)
            gbr[0, 0:512] = comb[0:512]          # z0 z1 r0 r1
            gbr[0, 512:768] = br[512:768]        # rh0 rh1
            gbr[0, 768:1024] = bi[512:768]       # xh0 xh1
            m["conv_b2"] = np.ascontiguousarray(
                conv_b.reshape(2, 128).T, np.float32)
            m["attn_b"] = attn_b.reshape(1, T).astype(ml_dtypes.bfloat16)
            m["gbias_row"] = gbr.astype(ml_dtypes.bfloat16)
        in_maps.append(m)

    res = run_bass_kernel_spmd(nc, in_maps, core_ids=list(range(NCORES)))
    outs = [res.results[c]["h_out"] for c in range(NCORES)]
    h = np.concatenate(outs, axis=0)  # [1024, 256]
    return h.reshape(B, LTMS, HH).astype(np.float32)


if __name__ == "__main__":
    nc = _get_nc(True)
    print("built ok")


# revision 3
# speedup vs baseline: 1.2338x; 1.0803x over previous
"""Trainium2 Bass kernel for nn_Attention_Encoder (conv1x1 -> time-softmax attention -> relu-GRU).

Sharding: pure data parallelism. 1024 segments split across 8 cores (S=128
per core); weights replicated. v2 redesign vs baseline:

phase A (per segment pair):
  x_T [C,2,T] via transpose-DMA; conv_T = relu(Wc^T x_T) (DVE TSP evac);
  conv_N obtained by PE transposes of the relu'd conv_T (bf16 PSUM
  pass-through, evacuated by a 2x-rate TensorCopy); scores -> exp (ACT,
  fused row-sum); x_att = E*conv on Pool (TT), then *rinv on DVE (4x TSP),
  stored [128, k, S, T] so writes are packed.

phase B: two software-pipelined chains of SC=64 segments (B half a step
  behind A) hide the recurrence latency. Per chain-step: one PSUM tile
  [128, 8, SC] holds z,r,rh,xh (concurrent per-bank accumulation groups,
  sim check skipped -- hardware zeroes only written bytes); ACT evacuates
  xh and runs one combined sigmoid over [z;r]; DVE: t1=r*rh(PSUM),
  q=t1+xh, hh=relu(q) (4x TSP); Pool: d=h-hh, e=z*d, h'=hh+e.
  GRU biases (generic path) are added via rank-1 matmuls into PSUM.
"""

import contextlib
import os
import sys

sys.path.insert(0, "/opt/trn_rl_repo")

import numpy as np
import ml_dtypes

import concourse.bass as bass
import concourse.tile as tile
from concourse import mybir
from concourse.bass_utils import run_bass_kernel_spmd

F32 = mybir.dt.float32
BF16 = mybir.dt.bfloat16
AF = mybir.ActivationFunctionType
OP = mybir.AluOpType

B, LTMS, TTS, C_IN, FF, HH = 64, 16, 256, 128, 256, 256
NCORES = 8
S = (B * LTMS) // NCORES  # 128 segments per core
T = TTS                   # 256 timesteps
SC = S // 2               # 64 segments per chain

# bfpack column layout (bf16): conv_w | attn_w | gru_w | gru_u | identity
BP_CW = 0
BP_AW = BP_CW + FF              # 256
BP_WG = BP_AW + 2 * T           # 768
BP_WU = BP_WG + 2 * 3 * HH      # 2304
BP_ID = BP_WU + 2 * 3 * HH      # 3840
BP_W = BP_ID + 128              # 3968


def build(zero_bias: bool) -> bass.Bass:
    nc = bass.Bass("TRN2", target_bir_lowering=False)

    x_d = nc.dram_tensor("x_shard", [S, T, C_IN], BF16, kind="ExternalInput")
    bp_d = nc.dram_tensor("bfpack", [128, BP_W], BF16, kind="ExternalInput")
    if not zero_bias:
        cb_d = nc.dram_tensor("conv_b2", [128, 2], F32, kind="ExternalInput")
        ab_d = nc.dram_tensor("attn_b", [1, T], BF16, kind="ExternalInput")
        # gru bias rows for rank-1 PSUM adds: [1, 8*128] bf16
        # order: z0 z1 r0 r1 (bi+br) | rh0 rh1 (br_h) | xh0 xh1 (bi_h)
        gb_d = nc.dram_tensor("gbias_row", [1, 8 * 128], BF16,
                              kind="ExternalInput")
    out_d = nc.dram_tensor("h_out", [S, HH], F32, kind="ExternalOutput")

    with tile.TileContext(nc, trace_sim=bool(os.environ.get("KTRACE"))) as tc:
        with contextlib.ExitStack() as ctx:
            singles = ctx.enter_context(tc.tile_pool(name="singles", bufs=1))

            bp_sb = singles.tile([128, BP_W], BF16)
            nc.sync.dma_start(bp_sb, bp_d[:])

            cw_sb = bp_sb[:, BP_CW:BP_CW + FF]
            aw_sb = bp_sb[:, BP_AW:BP_AW + 2 * T].rearrange(
                "p (k n) -> p k n", k=2)
            wg_sb = bp_sb[:, BP_WG:BP_WG + 1536].rearrange(
                "p (k n) -> p k n", k=2)
            wu_sb = bp_sb[:, BP_WU:BP_WU + 1536].rearrange(
                "p (k n) -> p k n", k=2)
            ident_bf = bp_sb[:, BP_ID:BP_ID + 128]

            # global x_att store: [F%128, F-chunk, S, T] bf16 (T packed)
            xatt = singles.tile([128, 2, S, T], BF16)

            if not zero_bias:
                cb_sb = singles.tile([128, 2], F32)
                nc.sync.dma_start(cb_sb, cb_d[:])
                ab_row = singles.tile([1, T], BF16)
                nc.sync.dma_start(ab_row, ab_d[:])
                gb_row = singles.tile([1, 8 * 128], BF16)
                nc.sync.dma_start(gb_row, gb_d[:])
                ones_col = singles.tile([1, 128], BF16)
                nc.vector.memset(ones_col, 1.0)
                ones_sc = ones_col[:, :SC]

            # ---------------- phase A ----------------
            apool = ctx.enter_context(tc.tile_pool(name="apool", bufs=3))
            with contextlib.ExitStack() as actx:
                apsum = actx.enter_context(
                    tc.tile_pool(name="apsum", bufs=1, space="PSUM"))

                # PE warmup: consume the weight-pack DMA on PE early
                # (borrows the ps_cn tag so phase A stays within 8 PSUM banks)
                ps_w1 = apsum.tile([128, 128], BF16, tag="ps_cn", bufs=2)
                nc.tensor.transpose(ps_w1, ident_bf, ident_bf)

                for s2 in range(S // 2):
                    s = 2 * s2
                    x_t = apool.tile([128, 2, T], BF16, tag="x_t", bufs=4)
                    nc.sync.dma_start_transpose(x_t[:, 0, :], x_d[s])
                    nc.sync.dma_start_transpose(x_t[:, 1, :], x_d[s + 1])

                    # conv_T = relu(W_c^T @ x_T): [F(2ch), seg, T]
                    ps_ct = apsum.tile([128, 2, 2, T], F32, tag="ps_ct",
                                       bufs=1)
                    for m in range(2):
                        nc.tensor.matmul(
                            ps_ct[:, m, :, :], cw_sb[:, bass.ts(m, 128)],
                            x_t, start=True, stop=True)
                    conv_t = apool.tile([128, 2, 2, T], BF16, tag="conv_t")
                    # evac+relu: chunk 0 on DVE; chunk 1 split DVE/ACT to
                    # balance the two engines across pairs
                    nc.vector.tensor_scalar(
                        conv_t[:, 0, :, :], ps_ct[:, 0, :, :],
                        0.0 if zero_bias else cb_sb[:, 0:1], 0.0,
                        OP.add, OP.max)
                    if False:
                        nc.scalar.activation(
                            conv_t[:, 1, :, :], ps_ct[:, 1, :, :], AF.Relu,
                            bias=0.0 if zero_bias else cb_sb[:, 1:2])
                    else:
                        nc.vector.tensor_scalar(
                            conv_t[:, 1, :, :], ps_ct[:, 1, :, :],
                            0.0 if zero_bias else cb_sb[:, 1:2], 0.0,
                            OP.add, OP.max)

                    # conv_N via PE transposes of relu'd conv_T (bf16 psum)
                    ps_cn = apsum.tile([128, 2, 2, FF], BF16, tag="ps_cn",
                                       bufs=2)
                    for seg in range(2):
                        for tch in range(2):
                            for m in range(2):
                                nc.tensor.transpose(
                                    ps_cn[:, seg, tch, bass.ts(m, 128)],
                                    conv_t[:, m, seg, bass.ts(tch, 128)],
                                    ident_bf)
                    conv_n = apool.tile([128, 2, 2, FF], BF16, tag="conv_n")
                    for seg in range(2):
                        nc.vector.tensor_copy(
                            conv_n[:, seg, :, :], ps_cn[:, seg, :, :])

                    # scores = conv_N^T @ A (+ b): [seg, F-ch, T]
                    ps_s = apsum.tile([128, 2, 2, T], F32, tag="ps_s", bufs=2)
                    for seg in range(2):
                        for m in range(2):
                            for k in range(2):
                                nc.tensor.matmul(
                                    ps_s[:, seg, m, :],
                                    conv_n[:, seg, k, bass.ts(m, 128)],
                                    aw_sb[:, k, :],
                                    start=(k == 0),
                                    stop=(k == 1) and zero_bias)
                            if not zero_bias:
                                nc.tensor.matmul(
                                    ps_s[:, seg, m, :], ones_col, ab_row,
                                    start=False, stop=True)

                    ee = apool.tile([128, 2, 2, T], BF16, tag="ee")
                    esum = apool.tile([128, 4], F32, tag="esum")
                    es4 = esum.rearrange("p (a b) -> p a b", a=2)
                    for seg in range(2):
                        for m in range(2):
                            nc.scalar.activation(
                                ee[:, seg, m, :], ps_s[:, seg, m, :], AF.Exp,
                                accum_out=es4[:, seg, m:m + 1])
                    rinv = apool.tile([128, 4], F32, tag="rinv")
                    nc.vector.reciprocal(rinv, esum)
                    ri4 = rinv.rearrange("p (a b) -> p a b", a=2)

                    # x_att[:, m, s+seg, :] = E * rinv * conv_T  (packed T)
                    # split: ec = E*conv on Pool (TT), then *rinv on DVE (4x)
                    ec = apool.tile([128, 2, 2, T], BF16, tag="ec")
                    for seg in range(2):
                        for m in range(2):
                            nc.gpsimd.tensor_mul(
                                ec[:, seg, m, :], ee[:, seg, m, :],
                                conv_t[:, m, seg, :])
                    for seg in range(2):
                        for m in range(2):
                            nc.vector.tensor_scalar_mul(
                                xatt[:, m, s + seg, :], ec[:, seg, m, :],
                                ri4[:, seg, m:m + 1])

            # ---------------- phase B: GRU over T steps, 2 chains ----------
            # gate columns in W/U: z=[0,256) m0,1 ; r=[256,512) m2,3 ;
            # h=[512,768) m4,5
            # psum tile layout [128, 8, SC]: z0 z1 r0 r1 | rh0 rh1 | xh0 xh1
            with contextlib.ExitStack() as bctx:
                hpool = bctx.enter_context(tc.tile_pool(name="hpool", bufs=2))
                gpool = bctx.enter_context(tc.tile_pool(name="gpool", bufs=3))
                bpsum = bctx.enter_context(
                    tc.tile_pool(name="bpsum", bufs=1, space="PSUM"))

                h_prev = [None, None]
                pend = [None, None]  # (t, ps, rz, xh_sb) awaiting elementwise

                def emit_pe_act(c, t):
                    """Matmuls + sigmoid + xh evac for (chain c, step t)."""
                    cb = c * SC
                    ps = bpsum.tile([128, 8, SC], F32, tag=f"ps{c}", bufs=3,
                                    name=f"ps{c}")
                    hp = h_prev[c]

                    # x-part matmuls (independent of h)
                    zr_stop = (t == 0) and zero_bias
                    for j, m in enumerate((0, 1)):      # z gates
                        for k in range(2):
                            nc.tensor.matmul(
                                ps[:, j, :], wg_sb[:, k, bass.ts(m, 128)],
                                xatt[:, k, cb:cb + SC, t],
                                start=(k == 0), stop=(k == 1) and zr_stop,
                                skip_group_check=True)
                    for j, m in enumerate((2, 3)):      # r gates
                        for k in range(2):
                            nc.tensor.matmul(
                                ps[:, 2 + j, :],
                                wg_sb[:, k, bass.ts(m, 128)],
                                xatt[:, k, cb:cb + SC, t],
                                start=(k == 0), stop=(k == 1) and zr_stop,
                                skip_group_check=True)
                    for j, m in enumerate((4, 5)):      # h gate (xh)
                        for k in range(2):
                            nc.tensor.matmul(
                                ps[:, 6 + j, :],
                                wg_sb[:, k, bass.ts(m, 128)],
                                xatt[:, k, cb:cb + SC, t],
                                start=(k == 0),
                                stop=(k == 1) and zero_bias,
                                skip_group_check=True)

                    if not zero_bias:
                        # rank-1 bias adds; z0..r1 into [0:4],
                        # xh into [6:8], rh (br_h) into [4:6]
                        for j in range(4):
                            nc.tensor.matmul(
                                ps[:, j, :], gb_row[:, bass.ts(j, 128)],
                                ones_sc, start=False, stop=(t == 0),
                                skip_group_check=True)
                        for j in range(2):
                            nc.tensor.matmul(
                                ps[:, 6 + j, :],
                                gb_row[:, bass.ts(6 + j, 128)],
                                ones_sc, start=False, stop=True,
                                skip_group_check=True)
                        for j in range(2):
                            nc.tensor.matmul(
                                ps[:, 4 + j, :],
                                gb_row[:, bass.ts(4 + j, 128)],
                                ones_sc, start=True, stop=(t == 0),
                                skip_group_check=True)

                    # ACT evacuates xh early (depends only on Wx)
                    xh_sb = gpool.tile([128, 2, SC], BF16, tag=f"xh{c}",
                                       bufs=2, name=f"xh{c}")
                    nc.scalar.copy(xh_sb, ps[:, 6:8, :])

                    rz = gpool.tile([128, 4, SC], BF16, tag=f"rz{c}",
                                    bufs=2, name=f"rz{c}")
                    if t > 0:
                        # U-part: r,z first (gate the sigmoid), then rh
                        for j, m in enumerate((2, 3)):
                            for k in range(2):
                                nc.tensor.matmul(
                                    ps[:, 2 + j, :],
                                    wu_sb[:, k, bass.ts(m, 128)],
                                    hp[:, k, :],
                                    start=False, stop=(k == 1),
                                    skip_group_check=True)
                        for j, m in enumerate((0, 1)):
                            for k in range(2):
                                nc.tensor.matmul(
                                    ps[:, j, :],
                                    wu_sb[:, k, bass.ts(m, 128)],
                                    hp[:, k, :],
                                    start=False, stop=(k == 1),
                                    skip_group_check=True)
                        for j, m in enumerate((4, 5)):  # rh
                            for k in range(2):
                                nc.tensor.matmul(
                                    ps[:, 4 + j, :],
                                    wu_sb[:, k, bass.ts(m, 128)],
                                    hp[:, k, :],
                                    start=(k == 0) and zero_bias,
                                    stop=(k == 1),
                                    skip_group_check=True)
                    # sigmoid over [z;r] in one ACT op
                    nc.scalar.activation(rz, ps[:, 0:4, :], AF.Sigmoid)
                    pend[c] = (t, ps, rz, xh_sb)

                def emit_dve(c):
                    """Elementwise chain for the pending (chain c) step.
                    The d/e blend ops run on Pool to cut DVE occupancy."""
                    t, ps, rz, xh_sb = pend[c]
                    hp = h_prev[c]
                    h_new = hpool.tile([128, 2, SC], BF16, tag=f"h{c}",
                                       name=f"h{c}")
                    hh = gpool.tile([128, 2, SC], BF16, tag=f"hh{c}",
                                    bufs=2, name=f"hh{c}")
                    d = gpool.tile([128, 2, SC], BF16, tag=f"d{c}",
                                   bufs=2, name=f"d{c}")
                    e = gpool.tile([128, 2, SC], BF16, tag=f"e{c}",
                                   bufs=2, name=f"e{c}")
                    have_rh = (t > 0) or not zero_bias
                    if have_rh:
                        t1 = gpool.tile([128, 2, SC], BF16, tag=f"t1{c}",
                                        bufs=2, name=f"t1{c}")
                        q = gpool.tile([128, 2, SC], BF16, tag=f"q{c}",
                                       bufs=2, name=f"q{c}")
                        nc.vector.tensor_mul(t1, rz[:, 2:4, :], ps[:, 4:6, :])
                        nc.vector.tensor_add(q, t1, xh_sb)
                    else:
                        q = xh_sb
                    # hh = relu(q) (TSP, 4x); then the z-blend
                    nc.vector.tensor_scalar_max(hh, q, 0.0)
                    if t > 0:
                        nc.gpsimd.tensor_sub(d, hp, hh)        # d = h - hh
                        nc.gpsimd.tensor_mul(e, rz[:, 0:2, :], d)
                        nc.gpsimd.tensor_add(h_new, hh, e)     # h' = hh+z(h-hh)
                    else:
                        nc.gpsimd.tensor_mul(e, rz[:, 0:2, :], hh)
                        nc.gpsimd.tensor_sub(h_new, hh, e)     # (1-z)*hh
                    h_prev[c] = h_new

                # software-pipelined: chain B runs half a step behind A.
                # Absolute-time pins phase-lock the two chains: a pin that's
                # already past is a no-op, so transient overruns self-correct.
                pin_base = float(os.environ.get("PIN_BASE", "228000"))  # inert unless PIN_P>0
                pin_p = float(os.environ.get("PIN_P", "0"))
                for t in range(T):
                    tp = pin_base + t * pin_p
                    with tc.tile_wait_until(tp / 1e6, enable=pin_p > 0):
                        emit_pe_act(0, t)
                    if t > 0:
                        with tc.tile_wait_until(tp / 1e6,
                                                enable=pin_p > 0):
                            emit_dve(1)
                    with tc.tile_wait_until((tp + 0.5 * pin_p) / 1e6,
                                            enable=pin_p > 0):
                        emit_pe_act(1, t)
                    with tc.tile_wait_until((tp + 0.46 * pin_p) / 1e6,
                                            enable=pin_p > 0):
                        emit_dve(0)
                emit_dve(1)

                # output: transpose h back to [S, H] and store fp32
                ps_o = bpsum.tile([64, 2, 2, 128], BF16, tag="ps_o", bufs=1)
                for c in range(2):
                    for ch in range(2):
                        nc.tensor.transpose(
                            ps_o[:, c, ch, :], h_prev[c][:, ch, :], ident_bf)
                out_sb = gpool.tile([64, 2, 2, 128], F32, tag="out_sb")
                nc.vector.tensor_copy(out_sb, ps_o)
                for c in range(2):
                    nc.sync.dma_start(
                        out_d[c * SC:(c + 1) * SC].rearrange(
                            "s (ch p) -> s ch p", ch=2), out_sb[:, c])

    _split_multi_waits(nc)
    return nc


def _split_multi_waits(nc: bass.Bass):
    """Encode at most ONE semaphore wait per ISA instruction: hoist extras
    onto preceding same-engine NoOp carriers."""
    fn = nc.m.functions[0]
    for blk in fn.blocks:
        insts = list(blk.instructions)
        out = []
        changed = False
        for inst in insts:
            si = inst.sync_info
            waits = list(si.on_wait) if si is not None else []
            if len(waits) > 1:
                changed = True
                for w in waits[:-1]:
                    out.append(mybir.InstNoOp(
                        name=f"I-wsplit-{nc.next_id()}",
                        engine=inst.engine,
                        ins=[], outs=[],
                        sync_info=mybir.SyncInfo(on_wait=[w], on_update=[]),
                    ))
                inst.sync_info = mybir.SyncInfo(
                    on_wait=[waits[-1]], on_update=list(si.on_update))
            out.append(inst)
        if changed:
            blk.instructions = out


_CACHE = {}


def _get_nc(zero_bias: bool) -> bass.Bass:
    if zero_bias not in _CACHE:
        _CACHE[zero_bias] = build(zero_bias)
    return _CACHE[zero_bias]


def _pack_weights(conv_w, attn_w, gru_w, gru_u):
    bf = ml_dtypes.bfloat16
    cw = (conv_w[0] if conv_w.ndim == 3 else conv_w).astype(bf)  # [128, 256]
    aw = attn_w.astype(bf).reshape(2, 128, T).transpose(1, 0, 2).reshape(
        128, 2 * T)
    wg = gru_w.astype(bf).reshape(2, 128, 768).transpose(1, 0, 2).reshape(
        128, 1536)
    wu = gru_u.astype(bf).reshape(2, 128, 768).transpose(1, 0, 2).reshape(
        128, 1536)
    ident = np.eye(128, dtype=np.float32).astype(bf)
    return np.ascontiguousarray(
        np.concatenate([cw, aw, wg, wu, ident], axis=1), bf)


def kernel(x, conv_w, conv_b, attn_w, attn_b, gru_w, gru_u, gru_b):
    x = np.asarray(x, dtype=np.float32)
    conv_w = np.asarray(conv_w, dtype=np.float32)
    conv_b = np.asarray(conv_b, dtype=np.float32)
    attn_w = np.asarray(attn_w, dtype=np.float32)
    attn_b = np.asarray(attn_b, dtype=np.float32)
    gru_w = np.asarray(gru_w, dtype=np.float32)
    gru_u = np.asarray(gru_u, dtype=np.float32)
    gru_b = np.asarray(gru_b, dtype=np.float32)

    zero_bias = (
        not conv_b.any() and not attn_b.any() and not gru_b.any())

    nc = _get_nc(zero_bias)

    xs_bf = x.reshape(B * LTMS, T, C_IN).astype(ml_dtypes.bfloat16)
    bfpack = _pack_weights(conv_w, attn_w, gru_w, gru_u)

    in_maps = []
    for c in range(NCORES):
        m = {
            "x_shard": np.ascontiguousarray(xs_bf[c * S: (c + 1) * S]),
            "bfpack": bfpack,
        }
        if not zero_bias:
            bi, br = gru_b[0], gru_b[1]
            comb = bi + br
            gbr = np.zeros((1, 8 * 128), np.float32)
            gbr[0, 0:512] = comb[0:512]          # z0 z1 r0 r1
            gbr[0, 512:768] = br[512:768]        # rh0 rh1
            gbr[0, 768:1024] = bi[512:768]       # xh0 xh1
            m["conv_b2"] = np.ascontiguousarray(
                conv_b.reshape(2, 128).T, np.float32)
            m["attn_b"] = attn_b.reshape(1, T).astype(ml_dtypes.bfloat16)
            m["gbias_row"] = gbr.astype(ml_dtypes.bfloat16)
        in_maps.append(m)

    res = run_bass_kernel_spmd(nc, in_maps, core_ids=list(range(NCORES)))
    outs = [res.results[c]["h_out"] for c in range(NCORES)]
    h = np.concatenate(outs, axis=0)  # [1024, 256]
    return h.reshape(B, LTMS, HH).astype(np.float32)


if __name__ == "__main__":
    nc = _get_nc(True)
    print("built ok")


# revision 4
# speedup vs baseline: 1.2476x; 1.0112x over previous
"""Trainium2 Bass kernel for nn_Attention_Encoder (conv1x1 -> time-softmax attention -> relu-GRU).

Sharding: pure data parallelism. 1024 segments split across 8 cores (S=128
per core); weights replicated. v2 redesign vs baseline:

phase A (per segment pair):
  x_T [C,2,T] via transpose-DMA; conv_T = relu(Wc^T x_T) (DVE TSP evac);
  conv_N obtained by PE transposes of the relu'd conv_T (bf16 PSUM
  pass-through, evacuated by a 2x-rate TensorCopy); scores -> exp (ACT,
  fused row-sum); x_att = E*conv on Pool (TT), then *rinv on DVE (4x TSP),
  stored [128, k, S, T] so writes are packed.

phase B: two software-pipelined chains of SC=64 segments (B half a step
  behind A) hide the recurrence latency. Per chain-step: one PSUM tile
  [128, 8, SC] holds z,r,rh,xh (concurrent per-bank accumulation groups,
  sim check skipped -- hardware zeroes only written bytes); ACT evacuates
  xh and runs one combined sigmoid over [z;r]; DVE: t1=r*rh(PSUM),
  q=t1+xh, hh=relu(q) (4x TSP); Pool: d=h-hh, e=z*d, h'=hh+e.
  GRU biases (generic path) are added via rank-1 matmuls into PSUM.
"""

import contextlib
import os
import sys

sys.path.insert(0, "/opt/trn_rl_repo")

import numpy as np
import ml_dtypes

import concourse.bass as bass
import concourse.tile as tile
from concourse import mybir
from concourse.bass_utils import run_bass_kernel_spmd

F32 = mybir.dt.float32
BF16 = mybir.dt.bfloat16
AF = mybir.ActivationFunctionType
OP = mybir.AluOpType

B, LTMS, TTS, C_IN, FF, HH = 64, 16, 256, 128, 256, 256
NCORES = 8
S = (B * LTMS) // NCORES  # 128 segments per core
T = TTS                   # 256 timesteps
SC = S // 2               # 64 segments per chain

# bfpack column layout (bf16): conv_w | attn_w | gru_w | gru_u | identity
BP_CW = 0
BP_AW = BP_CW + FF              # 256
BP_WG = BP_AW + 2 * T           # 768
BP_WU = BP_WG + 2 * 3 * HH      # 2304
BP_ID = BP_WU + 2 * 3 * HH      # 3840
BP_W = BP_ID + 128              # 3968


def build(zero_bias: bool) -> bass.Bass:
    nc = bass.Bass("TRN2", target_bir_lowering=False)

    x_d = nc.dram_tensor("x_shard", [S, T, C_IN], BF16, kind="ExternalInput")
    bp_d = nc.dram_tensor("bfpack", [128, BP_W], BF16, kind="ExternalInput")
    if not zero_bias:
        cb_d = nc.dram_tensor("conv_b2", [128, 2], F32, kind="ExternalInput")
        ab_d = nc.dram_tensor("attn_b", [1, T], BF16, kind="ExternalInput")
        # gru bias rows for rank-1 PSUM adds: [1, 8*128] bf16
        # order: z0 z1 r0 r1 (bi+br) | rh0 rh1 (br_h) | xh0 xh1 (bi_h)
        gb_d = nc.dram_tensor("gbias_row", [1, 8 * 128], BF16,
                              kind="ExternalInput")
    out_d = nc.dram_tensor("h_out", [S, HH], F32, kind="ExternalOutput")

    with tile.TileContext(nc, trace_sim=bool(os.environ.get("KTRACE"))) as tc:
        with contextlib.ExitStack() as ctx:
            singles = ctx.enter_context(tc.tile_pool(name="singles", bufs=1))

            bp_sb = singles.tile([128, BP_W], BF16)
            nc.sync.dma_start(bp_sb, bp_d[:])

            cw_sb = bp_sb[:, BP_CW:BP_CW + FF]
            aw_sb = bp_sb[:, BP_AW:BP_AW + 2 * T].rearrange(
                "p (k n) -> p k n", k=2)
            wg_sb = bp_sb[:, BP_WG:BP_WG + 1536].rearrange(
                "p (k n) -> p k n", k=2)
            wu_sb = bp_sb[:, BP_WU:BP_WU + 1536].rearrange(
                "p (k n) -> p k n", k=2)
            ident_bf = bp_sb[:, BP_ID:BP_ID + 128]

            # global x_att store: [F%128, F-chunk, S, T] bf16 (T packed)
            xatt = singles.tile([128, 2, S, T], BF16)

            if not zero_bias:
                cb_sb = singles.tile([128, 2], F32)
                nc.sync.dma_start(cb_sb, cb_d[:])
                ab_row = singles.tile([1, T], BF16)
                nc.sync.dma_start(ab_row, ab_d[:])
                gb_row = singles.tile([1, 8 * 128], BF16)
                nc.sync.dma_start(gb_row, gb_d[:])
                ones_col = singles.tile([1, 128], BF16)
                nc.vector.memset(ones_col, 1.0)
                ones_sc = ones_col[:, :SC]

            # ---------------- phase A ----------------
            apool = ctx.enter_context(tc.tile_pool(name="apool", bufs=3))
            with contextlib.ExitStack() as actx:
                apsum = actx.enter_context(
                    tc.tile_pool(name="apsum", bufs=1, space="PSUM"))

                # PE warmup: consume the weight-pack DMA on PE early
                # (borrows the ps_cn tag so phase A stays within 8 PSUM banks)
                ps_w1 = apsum.tile([128, 128], BF16, tag="ps_cn", bufs=2)
                nc.tensor.transpose(ps_w1, ident_bf, ident_bf)

                for s2 in range(S // 2):
                    s = 2 * s2
                    x_t = apool.tile([128, 2, T], BF16, tag="x_t", bufs=4)
                    nc.sync.dma_start_transpose(x_t[:, 0, :], x_d[s])
                    nc.sync.dma_start_transpose(x_t[:, 1, :], x_d[s + 1])

                    # conv_T = relu(W_c^T @ x_T): [F(2ch), seg, T]
                    ps_ct = apsum.tile([128, 2, 2, T], F32, tag="ps_ct",
                                       bufs=1)
                    for m in range(2):
                        nc.tensor.matmul(
                            ps_ct[:, m, :, :], cw_sb[:, bass.ts(m, 128)],
                            x_t, start=True, stop=True)
                    conv_t = apool.tile([128, 2, 2, T], BF16, tag="conv_t")
                    if zero_bias:
                        # single wide evac+relu amortizes the PSUM access
                        nc.vector.tensor_scalar_max(conv_t, ps_ct, 0.0)
                    else:
                        for mc in range(2):
                            nc.vector.tensor_scalar(
                                conv_t[:, mc, :, :], ps_ct[:, mc, :, :],
                                cb_sb[:, mc:mc + 1], 0.0, OP.add, OP.max)

                    # conv_N via PE transposes of relu'd conv_T (bf16 psum)
                    ps_cn = apsum.tile([128, 2, 2, FF], BF16, tag="ps_cn",
                                       bufs=2)
                    for seg in range(2):
                        for tch in range(2):
                            for m in range(2):
                                nc.tensor.transpose(
                                    ps_cn[:, seg, tch, bass.ts(m, 128)],
                                    conv_t[:, m, seg, bass.ts(tch, 128)],
                                    ident_bf)
                    conv_n = apool.tile([128, 2, 2, FF], BF16, tag="conv_n")
                    nc.vector.tensor_copy(conv_n, ps_cn)

                    # scores = conv_N^T @ A (+ b): [seg, F-ch, T]
                    ps_s = apsum.tile([128, 2, 2, T], F32, tag="ps_s", bufs=2)
                    for seg in range(2):
                        for m in range(2):
                            for k in range(2):
                                nc.tensor.matmul(
                                    ps_s[:, seg, m, :],
                                    conv_n[:, seg, k, bass.ts(m, 128)],
                                    aw_sb[:, k, :],
                                    start=(k == 0),
                                    stop=(k == 1) and zero_bias)
                            if not zero_bias:
                                nc.tensor.matmul(
                                    ps_s[:, seg, m, :], ones_col, ab_row,
                                    start=False, stop=True)

                    ee = apool.tile([128, 2, 2, T], BF16, tag="ee")
                    esum = apool.tile([128, 4], F32, tag="esum")
                    es4 = esum.rearrange("p (a b) -> p a b", a=2)
                    for seg in range(2):
                        for m in range(2):
                            nc.scalar.activation(
                                ee[:, seg, m, :], ps_s[:, seg, m, :], AF.Exp,
                                accum_out=es4[:, seg, m:m + 1])
                    rinv = apool.tile([128, 4], F32, tag="rinv")
                    nc.vector.reciprocal(rinv, esum)
                    ri4 = rinv.rearrange("p (a b) -> p a b", a=2)

                    # x_att[:, m, s+seg, :] = E * rinv * conv_T  (packed T)
                    # split: ec = E*conv on Pool (TT), then *rinv on DVE (4x)
                    ec = apool.tile([128, 2, 2, T], BF16, tag="ec")
                    for seg in range(2):
                        nc.gpsimd.tensor_mul(
                            ec[:, seg, :, :], ee[:, seg, :, :],
                            conv_t[:, :, seg, :])
                    for seg in range(2):
                        for m in range(2):
                            nc.vector.tensor_scalar_mul(
                                xatt[:, m, s + seg, :], ec[:, seg, m, :],
                                ri4[:, seg, m:m + 1])

            # ---------------- phase B: GRU over T steps, 2 chains ----------
            # gate columns in W/U: z=[0,256) m0,1 ; r=[256,512) m2,3 ;
            # h=[512,768) m4,5
            # psum tile layout [128, 8, SC]: z0 z1 r0 r1 | rh0 rh1 | xh0 xh1
            with contextlib.ExitStack() as bctx:
                hpool = bctx.enter_context(tc.tile_pool(name="hpool", bufs=2))
                gpool = bctx.enter_context(tc.tile_pool(name="gpool", bufs=3))
                bpsum = bctx.enter_context(
                    tc.tile_pool(name="bpsum", bufs=1, space="PSUM"))

                h_prev = [None, None]
                pend = [None, None]  # (t, ps, rz, xh_sb) awaiting elementwise

                def emit_pe_act(c, t):
                    """Matmuls + sigmoid + xh evac for (chain c, step t)."""
                    cb = c * SC
                    ps = bpsum.tile([128, 8, SC], F32, tag=f"ps{c}", bufs=3,
                                    name=f"ps{c}")
                    hp = h_prev[c]

                    # x-part matmuls (independent of h)
                    zr_stop = (t == 0) and zero_bias
                    for j, m in enumerate((0, 1)):      # z gates
                        for k in range(2):
                            nc.tensor.matmul(
                                ps[:, j, :], wg_sb[:, k, bass.ts(m, 128)],
                                xatt[:, k, cb:cb + SC, t],
                                start=(k == 0), stop=(k == 1) and zr_stop,
                                skip_group_check=True)
                    for j, m in enumerate((2, 3)):      # r gates
                        for k in range(2):
                            nc.tensor.matmul(
                                ps[:, 2 + j, :],
                                wg_sb[:, k, bass.ts(m, 128)],
                                xatt[:, k, cb:cb + SC, t],
                                start=(k == 0), stop=(k == 1) and zr_stop,
                                skip_group_check=True)
                    for j, m in enumerate((4, 5)):      # h gate (xh)
                        for k in range(2):
                            nc.tensor.matmul(
                                ps[:, 6 + j, :],
                                wg_sb[:, k, bass.ts(m, 128)],
                                xatt[:, k, cb:cb + SC, t],
                                start=(k == 0),
                                stop=(k == 1) and zero_bias,
                                skip_group_check=True)

                    if not zero_bias:
                        # rank-1 bias adds; z0..r1 into [0:4],
                        # xh into [6:8], rh (br_h) into [4:6]
                        for j in range(4):
                            nc.tensor.matmul(
                                ps[:, j, :], gb_row[:, bass.ts(j, 128)],
                                ones_sc, start=False, stop=(t == 0),
                                skip_group_check=True)
                        for j in range(2):
                            nc.tensor.matmul(
                                ps[:, 6 + j, :],
                                gb_row[:, bass.ts(6 + j, 128)],
                                ones_sc, start=False, stop=True,
                                skip_group_check=True)
                        for j in range(2):
                            nc.tensor.matmul(
                                ps[:, 4 + j, :],
                                gb_row[:, bass.ts(4 + j, 128)],
                                ones_sc, start=True, stop=(t == 0),
                                skip_group_check=True)

                    # ACT evacuates xh early (depends only on Wx)
                    xh_sb = gpool.tile([128, 2, SC], BF16, tag=f"xh{c}",
                                       bufs=2, name=f"xh{c}")
                    nc.scalar.copy(xh_sb, ps[:, 6:8, :])

                    rz = gpool.tile([128, 4, SC], BF16, tag=f"rz{c}",
                                    bufs=2, name=f"rz{c}")
                    if t > 0:
                        # U-part: r,z first (gate the sigmoid), then rh
                        for j, m in enumerate((2, 3)):
                            for k in range(2):
                                nc.tensor.matmul(
                                    ps[:, 2 + j, :],
                                    wu_sb[:, k, bass.ts(m, 128)],
                                    hp[:, k, :],
                                    start=False, stop=(k == 1),
                                    skip_group_check=True)
                        for j, m in enumerate((0, 1)):
                            for k in range(2):
                                nc.tensor.matmul(
                                    ps[:, j, :],
                                    wu_sb[:, k, bass.ts(m, 128)],
                                    hp[:, k, :],
                                    start=False, stop=(k == 1),
                                    skip_group_check=True)
                        for j, m in enumerate((4, 5)):  # rh
                            for k in range(2):
                                nc.tensor.matmul(
                                    ps[:, 4 + j, :],
                                    wu_sb[:, k, bass.ts(m, 128)],
                                    hp[:, k, :],
                                    start=(k == 0) and zero_bias,
                                    stop=(k == 1),
                                    skip_group_check=True)
                    # sigmoid over [z;r] in one ACT op
                    nc.scalar.activation(rz, ps[:, 0:4, :], AF.Sigmoid)
                    pend[c] = (t, ps, rz, xh_sb)

                def emit_dve(c):
                    """Elementwise chain for the pending (chain c) step.
                    The d/e blend ops run on Pool to cut DVE occupancy."""
                    t, ps, rz, xh_sb = pend[c]
                    hp = h_prev[c]
                    h_new = hpool.tile([128, 2, SC], BF16, tag=f"h{c}",
                                       name=f"h{c}")
                    hh = gpool.tile([128, 2, SC], BF16, tag=f"hh{c}",
                                    bufs=2, name=f"hh{c}")
                    d = gpool.tile([128, 2, SC], BF16, tag=f"d{c}",
                                   bufs=2, name=f"d{c}")
                    e = gpool.tile([128, 2, SC], BF16, tag=f"e{c}",
                                   bufs=2, name=f"e{c}")
                    have_rh = (t > 0) or not zero_bias
                    if have_rh:
                        t1 = gpool.tile([128, 2, SC], BF16, tag=f"t1{c}",
                                        bufs=2, name=f"t1{c}")
                        q = gpool.tile([128, 2, SC], BF16, tag=f"q{c}",
                                       bufs=2, name=f"q{c}")
                        nc.vector.tensor_mul(t1, rz[:, 2:4, :], ps[:, 4:6, :])
                        nc.vector.tensor_add(q, t1, xh_sb)
                    else:
                        q = xh_sb
                    # hh = relu(q) (TSP, 4x); then the z-blend
                    nc.vector.tensor_scalar_max(hh, q, 0.0)
                    if t > 0:
                        nc.gpsimd.tensor_sub(d, hp, hh)        # d = h - hh
                        nc.gpsimd.tensor_mul(e, rz[:, 0:2, :], d)
                        nc.gpsimd.tensor_add(h_new, hh, e)     # h' = hh+z(h-hh)
                    else:
                        nc.gpsimd.tensor_mul(e, rz[:, 0:2, :], hh)
                        nc.gpsimd.tensor_sub(h_new, hh, e)     # (1-z)*hh
                    h_prev[c] = h_new

                # software-pipelined: chain B runs half a step behind A.
                # Absolute-time pins phase-lock the two chains: a pin that's
                # already past is a no-op, so transient overruns self-correct.
                pin_base = float(os.environ.get("PIN_BASE", "228000"))  # inert unless PIN_P>0
                pin_p = float(os.environ.get("PIN_P", "0"))
                for t in range(T):
                    tp = pin_base + t * pin_p
                    with tc.tile_wait_until(tp / 1e6, enable=pin_p > 0):
                        emit_pe_act(0, t)
                    if t > 0:
                        with tc.tile_wait_until(tp / 1e6,
                                                enable=pin_p > 0):
                            emit_dve(1)
                    with tc.tile_wait_until((tp + 0.5 * pin_p) / 1e6,
                                            enable=pin_p > 0):
                        emit_pe_act(1, t)
                    with tc.tile_wait_until((tp + 0.46 * pin_p) / 1e6,
                                            enable=pin_p > 0):
                        emit_dve(0)
                emit_dve(1)

                # output: transpose h back to [S, H] and store fp32
                ps_o = bpsum.tile([64, 2, 2, 128], BF16, tag="ps_o", bufs=1)
                for c in range(2):
                    for ch in range(2):
                        nc.tensor.transpose(
                            ps_o[:, c, ch, :], h_prev[c][:, ch, :], ident_bf)
                out_sb = gpool.tile([64, 2, 2, 128], F32, tag="out_sb")
                nc.vector.tensor_copy(out_sb, ps_o)
                for c in range(2):
                    nc.sync.dma_start(
                        out_d[c * SC:(c + 1) * SC].rearrange(
                            "s (ch p) -> s ch p", ch=2), out_sb[:, c])

    _split_multi_waits(nc)
    return nc


def _split_multi_waits(nc: bass.Bass):
    """Encode at most ONE semaphore wait per ISA instruction: hoist extras
    onto preceding same-engine NoOp carriers."""
    fn = nc.m.functions[0]
    for blk in fn.blocks:
        insts = list(blk.instructions)
        out = []
        changed = False
        for inst in insts:
            si = inst.sync_info
            waits = list(si.on_wait) if si is not None else []
            if len(waits) > 1:
                changed = True
                for w in waits[:-1]:
                    out.append(mybir.InstNoOp(
                        name=f"I-wsplit-{nc.next_id()}",
                        engine=inst.engine,
                        ins=[], outs=[],
                        sync_info=mybir.SyncInfo(on_wait=[w], on_update=[]),
                    ))
                inst.sync_info = mybir.SyncInfo(
                    on_wait=[waits[-1]], on_update=list(si.on_update))
            out.append(inst)
        if changed:
            blk.instructions = out


_CACHE = {}


def _get_nc(zero_bias: bool) -> bass.Bass:
    if zero_bias not in _CACHE:
        _CACHE[zero_bias] = build(zero_bias)
    return _CACHE[zero_bias]


def _pack_weights(conv_w, attn_w, gru_w, gru_u):
    bf = ml_dtypes.bfloat16
    cw = (conv_w[0] if conv_w.ndim == 3 else conv_w).astype(bf)  # [128, 256]
    aw = attn_w.astype(bf).reshape(2, 128, T).transpose(1, 0, 2).reshape(
        128, 2 * T)
    wg = gru_w.astype(bf).reshape(2, 128, 768).transpose(1, 0, 2).reshape(
        128, 1536)
    wu = gru_u.astype(bf).reshape(2, 128, 768).transpose(1, 0, 2).reshape(
        128, 1536)
    ident = np.eye(128, dtype=np.float32).astype(bf)
    return np.ascontiguousarray(
        np.concatenate([cw, aw, wg, wu, ident], axis=1), bf)


def kernel(x, conv_w, conv_b, attn_w, attn_b, gru_w, gru_u, gru_b):
    x = np.asarray(x, dtype=np.float32)
    conv_w = np.asarray(conv_w, dtype=np.float32)
    conv_b = np.asarray(conv_b, dtype=np.float32)
    attn_w = np.asarray(attn_w, dtype=np.float32)
    attn_b = np.asarray(attn_b, dtype=np.float32)
    gru_w = np.asarray(gru_w, dtype=np.float32)
    gru_u = np.asarray(gru_u, dtype=np.float32)
    gru_b = np.asarray(gru_b, dtype=np.float32)

    zero_bias = (
        not conv_b.any() and not attn_b.any() and not gru_b.any())

    nc = _get_nc(zero_bias)

    xs_bf = x.reshape(B * LTMS, T, C_IN).astype(ml_dtypes.bfloat16)
    bfpack = _pack_weights(conv_w, attn_w, gru_w, gru_u)

    in_maps = []
    for c in range(NCORES):
        m = {
            "x_shard": np.ascontiguousarray(xs_bf[c * S: (c + 1) * S]),
            "bfpack": bfpack,
        }
        if not zero_bias:
            bi, br = gru_b[0], gru_b[1]
            comb = bi + br
            gbr = np.zeros((1, 8 * 128), np.float32)
            gbr[0, 0:512] = comb[0:512]          # z0 z1 r0 r1
            gbr[0, 512:768] = br[512:768]        # rh0 rh1
            gbr[0, 768:1024] = bi[512:768]       # xh0 xh1
            m["conv_b2"] = np.ascontiguousarray(
                conv_b.reshape(2, 128).T, np.float32)
            m["attn_b"] = attn_b.reshape(1, T).astype(ml_dtypes.bfloat16)
            m["gbias_row"] = gbr.astype(ml_dtypes.bfloat16)
        in_maps.append(m)

    res = run_bass_kernel_spmd(nc, in_maps, core_ids=list(range(NCORES)))
    outs = [res.results[c]["h_out"] for c in range(NCORES)]
    h = np.concatenate(outs, axis=0)  # [1024, 256]
    return h.reshape(B, LTMS, HH).astype(np.float32)


if __name__ == "__main__":
    nc = _get_nc(True)
    print("built ok")


# revision 5
# speedup vs baseline: 1.2812x; 1.0269x over previous
"""Trainium2 Bass kernel for nn_Attention_Encoder (conv1x1 -> time-softmax attention -> relu-GRU).

Sharding: pure data parallelism. 1024 segments split across 8 cores (S=128
per core); weights replicated. v2 redesign vs baseline:

phase A (per segment pair):
  x_T [C,2,T] via transpose-DMA; conv_T = relu(Wc^T x_T) (DVE TSP evac);
  conv_N obtained by PE transposes of the relu'd conv_T (bf16 PSUM
  pass-through, evacuated by a 2x-rate TensorCopy); scores -> exp (ACT,
  fused row-sum); x_att = E*conv on Pool (TT), then *rinv on DVE (4x TSP),
  stored [128, k, S, T] so writes are packed.

phase B: two software-pipelined chains of SC=64 segments (B half a step
  behind A) hide the recurrence latency. Per chain-step: one PSUM tile
  [128, 8, SC] holds z,r,rh,xh (concurrent per-bank accumulation groups,
  sim check skipped -- hardware zeroes only written bytes); ACT evacuates
  xh and runs one combined sigmoid over [z;r]; DVE: t1=r*rh(PSUM),
  q=t1+xh, hh=relu(q) (4x TSP); Pool: d=h-hh, e=z*d, h'=hh+e.
  GRU biases (generic path) are added via rank-1 matmuls into PSUM.
"""

import contextlib
import os
import sys

sys.path.insert(0, "/opt/trn_rl_repo")

import numpy as np
import ml_dtypes

import concourse.bass as bass
import concourse.tile as tile
from concourse import mybir
from concourse.bass_utils import run_bass_kernel_spmd

F32 = mybir.dt.float32
BF16 = mybir.dt.bfloat16
AF = mybir.ActivationFunctionType
OP = mybir.AluOpType

B, LTMS, TTS, C_IN, FF, HH = 64, 16, 256, 128, 256, 256
NCORES = 8
S = (B * LTMS) // NCORES  # 128 segments per core
T = TTS                   # 256 timesteps
SC = S // 2               # 64 segments per chain

# bfpack column layout (bf16): conv_w | attn_w | gru_w | gru_u | identity
BP_CW = 0
BP_AW = BP_CW + FF              # 256
BP_WG = BP_AW + 2 * T           # 768
BP_WU = BP_WG + 2 * 3 * HH      # 2304
BP_ID = BP_WU + 2 * 3 * HH      # 3840
BP_W = BP_ID + 128              # 3968


def build(zero_bias: bool) -> bass.Bass:
    nc = bass.Bass("TRN2", target_bir_lowering=False)

    x_d = nc.dram_tensor("x_shard", [S, T, C_IN], BF16, kind="ExternalInput")
    bp_d = nc.dram_tensor("bfpack", [128, BP_W], BF16, kind="ExternalInput")
    if not zero_bias:
        cb_d = nc.dram_tensor("conv_b2", [128, 2], F32, kind="ExternalInput")
        ab_d = nc.dram_tensor("attn_b", [1, T], BF16, kind="ExternalInput")
        # gru bias rows for rank-1 PSUM adds: [1, 8*128] bf16
        # order: z0 z1 r0 r1 (bi+br) | rh0 rh1 (br_h) | xh0 xh1 (bi_h)
        gb_d = nc.dram_tensor("gbias_row", [1, 8 * 128], BF16,
                              kind="ExternalInput")
    out_d = nc.dram_tensor("h_out", [S, HH], F32, kind="ExternalOutput")

    with tile.TileContext(nc, trace_sim=bool(os.environ.get("KTRACE"))) as tc:
        with contextlib.ExitStack() as ctx:
            singles = ctx.enter_context(tc.tile_pool(name="singles", bufs=1))

            bp_sb = singles.tile([128, BP_W], BF16)
            nc.sync.dma_start(bp_sb, bp_d[:])

            cw_sb = bp_sb[:, BP_CW:BP_CW + FF]
            aw_sb = bp_sb[:, BP_AW:BP_AW + 2 * T].rearrange(
                "p (k n) -> p k n", k=2)
            wg_sb = bp_sb[:, BP_WG:BP_WG + 1536].rearrange(
                "p (k n) -> p k n", k=2)
            wu_sb = bp_sb[:, BP_WU:BP_WU + 1536].rearrange(
                "p (k n) -> p k n", k=2)
            ident_bf = bp_sb[:, BP_ID:BP_ID + 128]

            # global x_att store: [F%128, F-chunk, S, T] bf16 (T packed)
            xatt = singles.tile([128, 2, S, T], BF16)

            if not zero_bias:
                cb_sb = singles.tile([128, 2], F32)
                nc.sync.dma_start(cb_sb, cb_d[:])
                ab_row = singles.tile([1, T], BF16)
                nc.sync.dma_start(ab_row, ab_d[:])
                gb_row = singles.tile([1, 8 * 128], BF16)
                nc.sync.dma_start(gb_row, gb_d[:])
                ones_col = singles.tile([1, 128], BF16)
                nc.vector.memset(ones_col, 1.0)
                ones_sc = ones_col[:, :SC]

            # ---------------- phase A ----------------
            apool = ctx.enter_context(tc.tile_pool(name="apool", bufs=3))
            with contextlib.ExitStack() as actx:
                apsum = actx.enter_context(
                    tc.tile_pool(name="apsum", bufs=1, space="PSUM"))

                # PE warmup: consume the weight-pack DMA on PE early
                # (borrows the ps_cn tag so phase A stays within 8 PSUM banks)
                ps_w1 = apsum.tile([128, 128], BF16, tag="ps_cn", bufs=2)
                nc.tensor.transpose(ps_w1, ident_bf, ident_bf)

                for s2 in range(S // 2):
                    s = 2 * s2
                    x_t = apool.tile([128, 2, T], BF16, tag="x_t", bufs=4)
                    nc.sync.dma_start_transpose(x_t[:, 0, :], x_d[s])
                    nc.sync.dma_start_transpose(x_t[:, 1, :], x_d[s + 1])

                    # conv_T = relu(W_c^T @ x_T): [F(2ch), seg, T]
                    ps_ct = apsum.tile([128, 2, 2, T], F32, tag="ps_ct",
                                       bufs=1)
                    for m in range(2):
                        nc.tensor.matmul(
                            ps_ct[:, m, :, :], cw_sb[:, bass.ts(m, 128)],
                            x_t, start=True, stop=True)
                    conv_t = apool.tile([128, 2, 2, T], BF16, tag="conv_t")
                    if zero_bias:
                        # single wide evac+relu amortizes the PSUM access
                        nc.vector.tensor_scalar_max(conv_t, ps_ct, 0.0)
                    else:
                        for mc in range(2):
                            nc.vector.tensor_scalar(
                                conv_t[:, mc, :, :], ps_ct[:, mc, :, :],
                                cb_sb[:, mc:mc + 1], 0.0, OP.add, OP.max)

                    # conv_N via PE transposes of relu'd conv_T (bf16 psum)
                    ps_cn = apsum.tile([128, 2, 2, FF], BF16, tag="ps_cn",
                                       bufs=2)
                    for seg in range(2):
                        for tch in range(2):
                            for m in range(2):
                                nc.tensor.transpose(
                                    ps_cn[:, seg, tch, bass.ts(m, 128)],
                                    conv_t[:, m, seg, bass.ts(tch, 128)],
                                    ident_bf)
                    conv_n = apool.tile([128, 2, 2, FF], BF16, tag="conv_n")
                    nc.vector.tensor_copy(conv_n, ps_cn)

                    # scores = conv_N^T @ A (+ b): [seg, F-ch, T]
                    ps_s = apsum.tile([128, 2, 2, T], F32, tag="ps_s", bufs=2)
                    for seg in range(2):
                        for m in range(2):
                            for k in range(2):
                                nc.tensor.matmul(
                                    ps_s[:, seg, m, :],
                                    conv_n[:, seg, k, bass.ts(m, 128)],
                                    aw_sb[:, k, :],
                                    start=(k == 0),
                                    stop=(k == 1) and zero_bias)
                            if not zero_bias:
                                nc.tensor.matmul(
                                    ps_s[:, seg, m, :], ones_col, ab_row,
                                    start=False, stop=True)

                    ee = apool.tile([128, 2, 2, T], BF16, tag="ee")
                    esum = apool.tile([128, 4], F32, tag="esum")
                    es4 = esum.rearrange("p (a b) -> p a b", a=2)
                    for seg in range(2):
                        for m in range(2):
                            nc.scalar.activation(
                                ee[:, seg, m, :], ps_s[:, seg, m, :], AF.Exp,
                                accum_out=es4[:, seg, m:m + 1])
                    rinv = apool.tile([128, 4], F32, tag="rinv")
                    nc.vector.reciprocal(rinv, esum)
                    ri4 = rinv.rearrange("p (a b) -> p a b", a=2)

                    # x_att[:, m, s+seg, :] = E * rinv * conv_T  (packed T)
                    # split: ec = E*conv on Pool (TT), then *rinv on DVE (4x)
                    ec = apool.tile([128, 2, 2, T], BF16, tag="ec")
                    for seg in range(2):
                        nc.gpsimd.tensor_mul(
                            ec[:, seg, :, :], ee[:, seg, :, :],
                            conv_t[:, :, seg, :])
                    for seg in range(2):
                        for m in range(2):
                            nc.vector.tensor_scalar_mul(
                                xatt[:, m, s + seg, :], ec[:, seg, m, :],
                                ri4[:, seg, m:m + 1])

            # ---------------- phase B: GRU over T steps, 2 chains ----------
            # gate columns in W/U: z=[0,256) m0,1 ; r=[256,512) m2,3 ;
            # h=[512,768) m4,5
            # psum tile layout [128, 8, SC]: z0 z1 r0 r1 | rh0 rh1 | xh0 xh1
            with contextlib.ExitStack() as bctx:
                hpool = bctx.enter_context(tc.tile_pool(name="hpool", bufs=2))
                gpool = bctx.enter_context(tc.tile_pool(name="gpool", bufs=3))
                bpsum = bctx.enter_context(
                    tc.tile_pool(name="bpsum", bufs=1, space="PSUM"))

                h_prev = [None, None]
                pend = [None, None]  # (t, ps, rz, xh_sb) awaiting elementwise

                def emit_pe_act(c, t):
                    """Matmuls + sigmoid + xh evac for (chain c, step t)."""
                    cb = c * SC
                    ps = bpsum.tile([128, 8, SC], F32, tag=f"ps{c}", bufs=3,
                                    name=f"ps{c}")
                    hp = h_prev[c]

                    # x-part matmuls (independent of h)
                    zr_stop = (t == 0) and zero_bias
                    for j, m in enumerate((0, 1)):      # z gates
                        for k in range(2):
                            nc.tensor.matmul(
                                ps[:, j, :], wg_sb[:, k, bass.ts(m, 128)],
                                xatt[:, k, cb:cb + SC, t],
                                start=(k == 0), stop=(k == 1) and zr_stop,
                                skip_group_check=True)
                    for j, m in enumerate((2, 3)):      # r gates
                        for k in range(2):
                            nc.tensor.matmul(
                                ps[:, 2 + j, :],
                                wg_sb[:, k, bass.ts(m, 128)],
                                xatt[:, k, cb:cb + SC, t],
                                start=(k == 0), stop=(k == 1) and zr_stop,
                                skip_group_check=True)
                    for j, m in enumerate((4, 5)):      # h gate (xh)
                        for k in range(2):
                            nc.tensor.matmul(
                                ps[:, 6 + j, :],
                                wg_sb[:, k, bass.ts(m, 128)],
                                xatt[:, k, cb:cb + SC, t],
                                start=(k == 0),
                                stop=(k == 1) and zero_bias,
                                skip_group_check=True)

                    if not zero_bias:
                        # rank-1 bias adds; z0..r1 into [0:4],
                        # xh into [6:8], rh (br_h) into [4:6]
                        for j in range(4):
                            nc.tensor.matmul(
                                ps[:, j, :], gb_row[:, bass.ts(j, 128)],
                                ones_sc, start=False, stop=(t == 0),
                                skip_group_check=True)
                        for j in range(2):
                            nc.tensor.matmul(
                                ps[:, 6 + j, :],
                                gb_row[:, bass.ts(6 + j, 128)],
                                ones_sc, start=False, stop=True,
                                skip_group_check=True)
                        for j in range(2):
                            nc.tensor.matmul(
                                ps[:, 4 + j, :],
                                gb_row[:, bass.ts(4 + j, 128)],
                                ones_sc, start=True, stop=(t == 0),
                                skip_group_check=True)

                    # ACT evacuates xh early (depends only on Wx)
                    xh_sb = gpool.tile([128, 2, SC], BF16, tag=f"xh{c}",
                                       bufs=2, name=f"xh{c}")
                    nc.scalar.copy(xh_sb, ps[:, 6:8, :])

                    rz = gpool.tile([128, 4, SC], BF16, tag=f"rz{c}",
                                    bufs=2, name=f"rz{c}")
                    if t > 0:
                        # U-part: r,z first (gate the sigmoid), then rh
                        for j, m in enumerate((2, 3)):
                            for k in range(2):
                                nc.tensor.matmul(
                                    ps[:, 2 + j, :],
                                    wu_sb[:, k, bass.ts(m, 128)],
                                    hp[:, k, :],
                                    start=False, stop=(k == 1),
                                    skip_group_check=True)
                        for j, m in enumerate((0, 1)):
                            for k in range(2):
                                nc.tensor.matmul(
                                    ps[:, j, :],
                                    wu_sb[:, k, bass.ts(m, 128)],
                                    hp[:, k, :],
                                    start=False, stop=(k == 1),
                                    skip_group_check=True)
                        for j, m in enumerate((4, 5)):  # rh
                            for k in range(2):
                                nc.tensor.matmul(
                                    ps[:, 4 + j, :],
                                    wu_sb[:, k, bass.ts(m, 128)],
                                    hp[:, k, :],
                                    start=(k == 0) and zero_bias,
                                    stop=(k == 1),
                                    skip_group_check=True)
                    # sigmoid over [z;r] in one ACT op
                    nc.scalar.activation(rz, ps[:, 0:4, :], AF.Sigmoid)
                    pend[c] = (t, ps, rz, xh_sb)

                def emit_dve(c):
                    """Elementwise chain for the pending (chain c) step.
                    The d/e blend ops run on Pool to cut DVE occupancy."""
                    t, ps, rz, xh_sb = pend[c]
                    hp = h_prev[c]
                    h_new = hpool.tile([128, 2, SC], BF16, tag=f"h{c}",
                                       name=f"h{c}")
                    hh = gpool.tile([128, 2, SC], BF16, tag=f"hh{c}",
                                    bufs=2, name=f"hh{c}")
                    d = gpool.tile([128, 2, SC], BF16, tag=f"d{c}",
                                   bufs=2, name=f"d{c}")
                    e = gpool.tile([128, 2, SC], BF16, tag=f"e{c}",
                                   bufs=2, name=f"e{c}")
                    have_rh = (t > 0) or not zero_bias
                    # off-chain: w = 1-z (DVE TSP 4x), m1 = z*h_prev (Pool)
                    w = gpool.tile([128, 2, SC], BF16, tag=f"w{c}",
                                   bufs=2, name=f"w{c}")
                    nc.vector.tensor_scalar(w, rz[:, 0:2, :], -1.0, 1.0,
                                            OP.mult, OP.add)
                    if t > 0:
                        m1 = gpool.tile([128, 2, SC], BF16, tag=f"m1{c}",
                                        bufs=2, name=f"m1{c}")
                        nc.gpsimd.tensor_mul(m1, rz[:, 0:2, :], hp)
                    if have_rh:
                        t1 = gpool.tile([128, 2, SC], BF16, tag=f"t1{c}",
                                        bufs=2, name=f"t1{c}")
                        q = gpool.tile([128, 2, SC], BF16, tag=f"q{c}",
                                       bufs=2, name=f"q{c}")
                        nc.vector.tensor_mul(t1, rz[:, 2:4, :], ps[:, 4:6, :])
                        nc.vector.tensor_add(q, t1, xh_sb)
                    else:
                        q = xh_sb
                    # hh = relu(q) (TSP, 4x); then only 2 Pool ops on-chain:
                    # h' = w*hh + m1
                    nc.vector.tensor_scalar_max(hh, q, 0.0)
                    if t > 0:
                        nc.gpsimd.tensor_mul(e, w, hh)         # e = (1-z)*hh
                        nc.gpsimd.tensor_add(h_new, e, m1)
                    else:
                        nc.gpsimd.tensor_mul(h_new, w, hh)     # h0 = 0
                    h_prev[c] = h_new

                # software-pipelined: chain B runs half a step behind A.
                # Absolute-time pins phase-lock the two chains: a pin that's
                # already past is a no-op, so transient overruns self-correct.
                pin_base = float(os.environ.get("PIN_BASE", "228000"))  # inert unless PIN_P>0
                pin_p = float(os.environ.get("PIN_P", "0"))
                for t in range(T):
                    tp = pin_base + t * pin_p
                    with tc.tile_wait_until(tp / 1e6, enable=pin_p > 0):
                        emit_pe_act(0, t)
                    if t > 0:
                        with tc.tile_wait_until(tp / 1e6,
                                                enable=pin_p > 0):
                            emit_dve(1)
                    with tc.tile_wait_until((tp + 0.5 * pin_p) / 1e6,
                                            enable=pin_p > 0):
                        emit_pe_act(1, t)
                    with tc.tile_wait_until((tp + 0.46 * pin_p) / 1e6,
                                            enable=pin_p > 0):
                        emit_dve(0)
                emit_dve(1)

                # output: transpose h back to [S, H] and store fp32
                ps_o = bpsum.tile([64, 2, 2, 128], BF16, tag="ps_o", bufs=1)
                for c in range(2):
                    for ch in range(2):
                        nc.tensor.transpose(
                            ps_o[:, c, ch, :], h_prev[c][:, ch, :], ident_bf)
                out_sb = gpool.tile([64, 2, 2, 128], F32, tag="out_sb")
                nc.vector.tensor_copy(out_sb, ps_o)
                for c in range(2):
                    nc.sync.dma_start(
                        out_d[c * SC:(c + 1) * SC].rearrange(
                            "s (ch p) -> s ch p", ch=2), out_sb[:, c])

    _split_multi_waits(nc)
    return nc


def _split_multi_waits(nc: bass.Bass):
    """Encode at most ONE semaphore wait per ISA instruction: hoist extras
    onto preceding same-engine NoOp carriers."""
    fn = nc.m.functions[0]
    for blk in fn.blocks:
        insts = list(blk.instructions)
        out = []
        changed = False
        for inst in insts:
            si = inst.sync_info
            waits = list(si.on_wait) if si is not None else []
            if len(waits) > 1:
                changed = True
                for w in waits[:-1]:
                    out.append(mybir.InstNoOp(
                        name=f"I-wsplit-{nc.next_id()}",
                        engine=inst.engine,
                        ins=[], outs=[],
                        sync_info=mybir.SyncInfo(on_wait=[w], on_update=[]),
                    ))
                inst.sync_info = mybir.SyncInfo(
                    on_wait=[waits[-1]], on_update=list(si.on_update))
            out.append(inst)
        if changed:
            blk.instructions = out


_CACHE = {}


def _get_nc(zero_bias: bool) -> bass.Bass:
    if zero_bias not in _CACHE:
        _CACHE[zero_bias] = build(zero_bias)
    return _CACHE[zero_bias]


def _pack_weights(conv_w, attn_w, gru_w, gru_u):
    bf = ml_dtypes.bfloat16
    cw = (conv_w[0] if conv_w.ndim == 3 else conv_w).astype(bf)  # [128, 256]
    aw = attn_w.astype(bf).reshape(2, 128, T).transpose(1, 0, 2).reshape(
        128, 2 * T)
    wg = gru_w.astype(bf).reshape(2, 128, 768).transpose(1, 0, 2).reshape(
        128, 1536)
    wu = gru_u.astype(bf).reshape(2, 128, 768).transpose(1, 0, 2).reshape(
        128, 1536)
    ident = np.eye(128, dtype=np.float32).astype(bf)
    return np.ascontiguousarray(
        np.concatenate([cw, aw, wg, wu, ident], axis=1), bf)


def kernel(x, conv_w, conv_b, attn_w, attn_b, gru_w, gru_u, gru_b):
    x = np.asarray(x, dtype=np.float32)
    conv_w = np.asarray(conv_w, dtype=np.float32)
    conv_b = np.asarray(conv_b, dtype=np.float32)
    attn_w = np.asarray(attn_w, dtype=np.float32)
    attn_b = np.asarray(attn_b, dtype=np.float32)
    gru_w = np.asarray(gru_w, dtype=np.float32)
    gru_u = np.asarray(gru_u, dtype=np.float32)
    gru_b = np.asarray(gru_b, dtype=np.float32)

    zero_bias = (
        not conv_b.any() and not attn_b.any() and not gru_b.any())

    nc = _get_nc(zero_bias)

    xs_bf = x.reshape(B * LTMS, T, C_IN).astype(ml_dtypes.bfloat16)
    bfpack = _pack_weights(conv_w, attn_w, gru_w, gru_u)

    in_maps = []
    for c in range(NCORES):
        m = {
            "x_shard": np.ascontiguousarray(xs_bf[c * S: (c + 1) * S]),
            "bfpack": bfpack,
        }
        if not zero_bias:
            bi, br = gru_b[0], gru_b[1]
            comb = bi + br
            gbr = np.zeros((1, 8 * 128), np.float32)
            gbr[0, 0:512] = comb[0:512]          # z0 z1 r0 r1
            gbr[0, 512:768] = br[512:768]        # rh0 rh1
            gbr[0, 768:1024] = bi[512:768]       # xh0 xh1
            m["conv_b2"] = np.ascontiguousarray(
                conv_b.reshape(2, 128).T, np.float32)
            m["attn_b"] = attn_b.reshape(1, T).astype(ml_dtypes.bfloat16)
            m["gbias_row"] = gbr.astype(ml_dtypes.bfloat16)
        in_maps.append(m)

    res = run_bass_kernel_spmd(nc, in_maps, core_ids=list(range(NCORES)))
    outs = [res.results[c]["h_out"] for c in range(NCORES)]
    h = np.concatenate(outs, axis=0)  # [1024, 256]
    return h.reshape(B, LTMS, HH).astype(np.float32)


if __name__ == "__main__":
    nc = _get_nc(True)
    print("built ok")


# revision 6
# speedup vs baseline: 1.3230x; 1.0326x over previous
"""Trainium2 Bass kernel for nn_Attention_Encoder (conv1x1 -> time-softmax attention -> relu-GRU).

Sharding: pure data parallelism. 1024 segments split across 8 cores (S=128
per core); weights replicated. v2 redesign vs baseline:

phase A (per segment pair):
  x_T [C,2,T] via transpose-DMA; conv_T = relu(Wc^T x_T) (DVE TSP evac);
  conv_N obtained by PE transposes of the relu'd conv_T (bf16 PSUM
  pass-through, evacuated by a 2x-rate TensorCopy); scores -> exp (ACT,
  fused row-sum); x_att = E*conv on Pool (TT), then *rinv on DVE (4x TSP),
  stored [128, k, S, T] so writes are packed.

phase B: two software-pipelined chains of SC=64 segments (B half a step
  behind A) hide the recurrence latency. Per chain-step: one PSUM tile
  [128, 8, SC] holds z,r,rh,xh (concurrent per-bank accumulation groups,
  sim check skipped -- hardware zeroes only written bytes); ACT evacuates
  xh and runs one combined sigmoid over [z;r]; DVE: t1=r*rh(PSUM),
  q=t1+xh, hh=relu(q) (4x TSP); Pool: d=h-hh, e=z*d, h'=hh+e.
  GRU biases (generic path) are added via rank-1 matmuls into PSUM.
"""

import contextlib
import os
import sys

sys.path.insert(0, "/opt/trn_rl_repo")

import numpy as np
import ml_dtypes

import concourse.bass as bass
import concourse.tile as tile
from concourse import mybir
from concourse.bass_utils import run_bass_kernel_spmd

F32 = mybir.dt.float32
BF16 = mybir.dt.bfloat16
AF = mybir.ActivationFunctionType
OP = mybir.AluOpType

B, LTMS, TTS, C_IN, FF, HH = 64, 16, 256, 128, 256, 256
NCORES = 8
S = (B * LTMS) // NCORES  # 128 segments per core
T = TTS                   # 256 timesteps
SC = S // 2               # 64 segments per chain

# bfpack column layout (bf16): conv_w | attn_w | gru_w | gru_u | identity
BP_CW = 0
BP_AW = BP_CW + FF              # 256
BP_WG = BP_AW + 2 * T           # 768
BP_WU = BP_WG + 2 * 3 * HH      # 2304
BP_ID = BP_WU + 2 * 3 * HH      # 3840
BP_W = BP_ID + 128              # 3968


def build(zero_bias: bool) -> bass.Bass:
    nc = bass.Bass("TRN2", target_bir_lowering=False)

    x_d = nc.dram_tensor("x_shard", [S, T, C_IN], BF16, kind="ExternalInput")
    bp_d = nc.dram_tensor("bfpack", [128, BP_W], BF16, kind="ExternalInput")
    if not zero_bias:
        cb_d = nc.dram_tensor("conv_b2", [128, 2], F32, kind="ExternalInput")
        ab_d = nc.dram_tensor("attn_b", [1, T], BF16, kind="ExternalInput")
        # gru bias rows for rank-1 PSUM adds: [1, 8*128] bf16
        # order: z0 z1 r0 r1 (bi+br) | rh0 rh1 (br_h) | xh0 xh1 (bi_h)
        gb_d = nc.dram_tensor("gbias_row", [1, 8 * 128], BF16,
                              kind="ExternalInput")
    out_d = nc.dram_tensor("h_out", [S, HH], F32, kind="ExternalOutput")

    with tile.TileContext(nc, trace_sim=bool(os.environ.get("KTRACE"))) as tc:
        with contextlib.ExitStack() as ctx:
            singles = ctx.enter_context(tc.tile_pool(name="singles", bufs=1))

            bp_sb = singles.tile([128, BP_W], BF16)
            nc.sync.dma_start(bp_sb, bp_d[:])

            cw_sb = bp_sb[:, BP_CW:BP_CW + FF]
            aw_sb = bp_sb[:, BP_AW:BP_AW + 2 * T].rearrange(
                "p (k n) -> p k n", k=2)
            wg_sb = bp_sb[:, BP_WG:BP_WG + 1536].rearrange(
                "p (k n) -> p k n", k=2)
            wu_sb = bp_sb[:, BP_WU:BP_WU + 1536].rearrange(
                "p (k n) -> p k n", k=2)
            ident_bf = bp_sb[:, BP_ID:BP_ID + 128]

            # global x_att store: [F%128, F-chunk, S, T] bf16 (T packed)
            xatt = singles.tile([128, 2, S, T], BF16)

            if not zero_bias:
                cb_sb = singles.tile([128, 2], F32)
                nc.sync.dma_start(cb_sb, cb_d[:])
                ab_row = singles.tile([1, T], BF16)
                nc.sync.dma_start(ab_row, ab_d[:])
                gb_row = singles.tile([1, 8 * 128], BF16)
                nc.sync.dma_start(gb_row, gb_d[:])
                ones_col = singles.tile([1, 128], BF16)
                nc.vector.memset(ones_col, 1.0)
                ones_sc = ones_col[:, :SC]

            # ---------------- phase A ----------------
            apool = ctx.enter_context(tc.tile_pool(name="apool", bufs=3))
            with contextlib.ExitStack() as actx:
                apsum = actx.enter_context(
                    tc.tile_pool(name="apsum", bufs=1, space="PSUM"))

                # PE warmup: consume the weight-pack DMA on PE early
                # (borrows the ps_cn tag so phase A stays within 8 PSUM banks)
                ps_w1 = apsum.tile([128, 128], BF16, tag="ps_cn", bufs=2)
                nc.tensor.transpose(ps_w1, ident_bf, ident_bf)

                for s2 in range(S // 2):
                    s = 2 * s2
                    x_t = apool.tile([128, 2, T], BF16, tag="x_t", bufs=4)
                    nc.sync.dma_start_transpose(x_t[:, 0, :], x_d[s])
                    nc.sync.dma_start_transpose(x_t[:, 1, :], x_d[s + 1])

                    # conv_T = relu(W_c^T @ x_T): [F(2ch), seg, T]
                    ps_ct = apsum.tile([128, 2, 2, T], F32, tag="ps_ct",
                                       bufs=1)
                    for m in range(2):
                        nc.tensor.matmul(
                            ps_ct[:, m, :, :], cw_sb[:, bass.ts(m, 128)],
                            x_t, start=True, stop=True)
                    conv_t = apool.tile([128, 2, 2, T], BF16, tag="conv_t")
                    if zero_bias:
                        # single wide evac+relu amortizes the PSUM access
                        nc.vector.tensor_scalar_max(conv_t, ps_ct, 0.0)
                    else:
                        for mc in range(2):
                            nc.vector.tensor_scalar(
                                conv_t[:, mc, :, :], ps_ct[:, mc, :, :],
                                cb_sb[:, mc:mc + 1], 0.0, OP.add, OP.max)

                    # conv_N via PE transposes of relu'd conv_T (bf16 psum)
                    ps_cn = apsum.tile([128, 2, 2, FF], BF16, tag="ps_cn",
                                       bufs=2)
                    for seg in range(2):
                        for tch in range(2):
                            for m in range(2):
                                nc.tensor.transpose(
                                    ps_cn[:, seg, tch, bass.ts(m, 128)],
                                    conv_t[:, m, seg, bass.ts(tch, 128)],
                                    ident_bf)
                    conv_n = apool.tile([128, 2, 2, FF], BF16, tag="conv_n")
                    nc.vector.tensor_copy(conv_n, ps_cn)

                    # scores = conv_N^T @ A (+ b): [seg, F-ch, T]
                    ps_s = apsum.tile([128, 2, 2, T], F32, tag="ps_s", bufs=2)
                    for seg in range(2):
                        for m in range(2):
                            for k in range(2):
                                nc.tensor.matmul(
                                    ps_s[:, seg, m, :],
                                    conv_n[:, seg, k, bass.ts(m, 128)],
                                    aw_sb[:, k, :],
                                    start=(k == 0),
                                    stop=(k == 1) and zero_bias)
                            if not zero_bias:
                                nc.tensor.matmul(
                                    ps_s[:, seg, m, :], ones_col, ab_row,
                                    start=False, stop=True)

                    ee = apool.tile([128, 2, 2, T], BF16, tag="ee")
                    esum = apool.tile([128, 4], F32, tag="esum")
                    es4 = esum.rearrange("p (a b) -> p a b", a=2)
                    for seg in range(2):
                        for m in range(2):
                            nc.scalar.activation(
                                ee[:, seg, m, :], ps_s[:, seg, m, :], AF.Exp,
                                accum_out=es4[:, seg, m:m + 1])
                    rinv = apool.tile([128, 4], F32, tag="rinv")
                    nc.vector.reciprocal(rinv, esum)
                    ri4 = rinv.rearrange("p (a b) -> p a b", a=2)

                    # x_att[:, m, s+seg, :] = E * rinv * conv_T  (packed T)
                    # split: ec = E*conv on Pool (TT), then *rinv on DVE (4x)
                    ec = apool.tile([128, 2, 2, T], BF16, tag="ec")
                    for seg in range(2):
                        nc.gpsimd.tensor_mul(
                            ec[:, seg, :, :], ee[:, seg, :, :],
                            conv_t[:, :, seg, :])
                    for seg in range(2):
                        for m in range(2):
                            nc.vector.tensor_scalar_mul(
                                xatt[:, m, s + seg, :], ec[:, seg, m, :],
                                ri4[:, seg, m:m + 1])

            # ---------------- phase B: GRU over T steps, 2 chains ----------
            # gate columns in W/U: z=[0,256) m0,1 ; r=[256,512) m2,3 ;
            # h=[512,768) m4,5
            # psum tile layout [128, 8, SC]: z0 z1 r0 r1 | rh0 rh1 | xh0 xh1
            with contextlib.ExitStack() as bctx:
                hpool = bctx.enter_context(tc.tile_pool(name="hpool", bufs=2))
                gpool = bctx.enter_context(tc.tile_pool(name="gpool", bufs=3))
                bpsum = bctx.enter_context(
                    tc.tile_pool(name="bpsum", bufs=1, space="PSUM"))

                h_prev = [None, None]
                pend = [None, None]  # (t, ps, rz, xh_sb) awaiting elementwise

                def emit_pe_act(c, t):
                    """Matmuls + sigmoid + xh evac for (chain c, step t)."""
                    cb = c * SC
                    ps = bpsum.tile([128, 8, SC], F32, tag=f"ps{c}", bufs=3,
                                    name=f"ps{c}")
                    hp = h_prev[c]

                    # x-part matmuls (independent of h)
                    zr_stop = (t == 0) and zero_bias
                    for j, m in enumerate((0, 1)):      # z gates
                        for k in range(2):
                            nc.tensor.matmul(
                                ps[:, j, :], wg_sb[:, k, bass.ts(m, 128)],
                                xatt[:, k, cb:cb + SC, t],
                                start=(k == 0), stop=(k == 1) and zr_stop,
                                skip_group_check=True)
                    for j, m in enumerate((2, 3)):      # r gates
                        for k in range(2):
                            nc.tensor.matmul(
                                ps[:, 2 + j, :],
                                wg_sb[:, k, bass.ts(m, 128)],
                                xatt[:, k, cb:cb + SC, t],
                                start=(k == 0), stop=(k == 1) and zr_stop,
                                skip_group_check=True)
                    for j, m in enumerate((4, 5)):      # h gate (xh)
                        for k in range(2):
                            nc.tensor.matmul(
                                ps[:, 6 + j, :],
                                wg_sb[:, k, bass.ts(m, 128)],
                                xatt[:, k, cb:cb + SC, t],
                                start=(k == 0),
                                stop=(k == 1) and zero_bias,
                                skip_group_check=True)

                    if not zero_bias:
                        # rank-1 bias adds; z0..r1 into [0:4],
                        # xh into [6:8], rh (br_h) into [4:6]
                        for j in range(4):
                            nc.tensor.matmul(
                                ps[:, j, :], gb_row[:, bass.ts(j, 128)],
                                ones_sc, start=False, stop=(t == 0),
                                skip_group_check=True)
                        for j in range(2):
                            nc.tensor.matmul(
                                ps[:, 6 + j, :],
                                gb_row[:, bass.ts(6 + j, 128)],
                                ones_sc, start=False, stop=True,
                                skip_group_check=True)
                        for j in range(2):
                            nc.tensor.matmul(
                                ps[:, 4 + j, :],
                                gb_row[:, bass.ts(4 + j, 128)],
                                ones_sc, start=True, stop=(t == 0),
                                skip_group_check=True)

                    # ACT evacuates xh early (depends only on Wx)
                    xh_sb = gpool.tile([128, 2, SC], BF16, tag=f"xh{c}",
                                       bufs=2, name=f"xh{c}")
                    nc.scalar.copy(xh_sb, ps[:, 6:8, :])

                    rz = gpool.tile([128, 4, SC], BF16, tag=f"rz{c}",
                                    bufs=2, name=f"rz{c}")
                    if t > 0:
                        # U-part: r,z first (gate the sigmoid), then rh
                        for j, m in enumerate((2, 3)):
                            for k in range(2):
                                nc.tensor.matmul(
                                    ps[:, 2 + j, :],
                                    wu_sb[:, k, bass.ts(m, 128)],
                                    hp[:, k, :],
                                    start=False, stop=(k == 1),
                                    skip_group_check=True)
                        for j, m in enumerate((0, 1)):
                            for k in range(2):
                                nc.tensor.matmul(
                                    ps[:, j, :],
                                    wu_sb[:, k, bass.ts(m, 128)],
                                    hp[:, k, :],
                                    start=False, stop=(k == 1),
                                    skip_group_check=True)
                        for j, m in enumerate((4, 5)):  # rh
                            for k in range(2):
                                nc.tensor.matmul(
                                    ps[:, 4 + j, :],
                                    wu_sb[:, k, bass.ts(m, 128)],
                                    hp[:, k, :],
                                    start=(k == 0) and zero_bias,
                                    stop=(k == 1),
                                    skip_group_check=True)
                    # sigmoid over [z;r] in one ACT op
                    nc.scalar.activation(rz, ps[:, 0:4, :], AF.Sigmoid)
                    pend[c] = (t, ps, rz, xh_sb)

                def emit_dve(c):
                    """Elementwise chain for the pending (chain c) step.
                    The d/e blend ops run on Pool to cut DVE occupancy."""
                    t, ps, rz, xh_sb = pend[c]
                    hp = h_prev[c]
                    h_new = hpool.tile([128, 2, SC], BF16, tag=f"h{c}",
                                       name=f"h{c}")
                    hh = gpool.tile([128, 2, SC], BF16, tag=f"hh{c}",
                                    bufs=2, name=f"hh{c}")
                    d = gpool.tile([128, 2, SC], BF16, tag=f"d{c}",
                                   bufs=2, name=f"d{c}")
                    e = gpool.tile([128, 2, SC], BF16, tag=f"e{c}",
                                   bufs=2, name=f"e{c}")
                    have_rh = (t > 0) or not zero_bias
                    # rz[:,0:2] is already w = 1-z (z weights negated on
                    # host). Off-chain on Pool: m1 = z*h = h - w*h
                    w = rz[:, 0:2, :]
                    if t > 0:
                        mw = gpool.tile([128, 2, SC], BF16, tag=f"mw{c}",
                                        bufs=2, name=f"mw{c}")
                        m1 = gpool.tile([128, 2, SC], BF16, tag=f"m1{c}",
                                        bufs=2, name=f"m1{c}")
                        nc.gpsimd.tensor_mul(mw, w, hp)
                        nc.gpsimd.tensor_sub(m1, hp, mw)
                    if have_rh:
                        t1 = gpool.tile([128, 2, SC], BF16, tag=f"t1{c}",
                                        bufs=2, name=f"t1{c}")
                        q = gpool.tile([128, 2, SC], BF16, tag=f"q{c}",
                                       bufs=2, name=f"q{c}")
                        nc.vector.tensor_mul(t1, rz[:, 2:4, :], ps[:, 4:6, :])
                        nc.vector.tensor_add(q, t1, xh_sb)
                    else:
                        q = xh_sb
                    # hh = relu(q) (TSP, 4x); then only 2 Pool ops on-chain:
                    # h' = w*hh + m1
                    nc.vector.tensor_scalar_max(hh, q, 0.0)
                    if t > 0:
                        nc.gpsimd.tensor_mul(e, w, hh)         # e = (1-z)*hh
                        nc.gpsimd.tensor_add(h_new, e, m1)
                    else:
                        nc.gpsimd.tensor_mul(h_new, w, hh)     # h0 = 0
                    h_prev[c] = h_new

                # software-pipelined: chain B runs half a step behind A.
                # Absolute-time pins phase-lock the two chains: a pin that's
                # already past is a no-op, so transient overruns self-correct.
                pin_base = float(os.environ.get("PIN_BASE", "228000"))  # inert unless PIN_P>0
                pin_p = float(os.environ.get("PIN_P", "0"))
                for t in range(T):
                    tp = pin_base + t * pin_p
                    with tc.tile_wait_until(tp / 1e6, enable=pin_p > 0):
                        emit_pe_act(0, t)
                    if t > 0:
                        with tc.tile_wait_until(tp / 1e6,
                                                enable=pin_p > 0):
                            emit_dve(1)
                    with tc.tile_wait_until((tp + 0.5 * pin_p) / 1e6,
                                            enable=pin_p > 0):
                        emit_pe_act(1, t)
                    with tc.tile_wait_until((tp + 0.46 * pin_p) / 1e6,
                                            enable=pin_p > 0):
                        emit_dve(0)
                emit_dve(1)

                # output: transpose h back to [S, H] and store fp32
                ps_o = bpsum.tile([64, 2, 2, 128], BF16, tag="ps_o", bufs=1)
                for c in range(2):
                    for ch in range(2):
                        nc.tensor.transpose(
                            ps_o[:, c, ch, :], h_prev[c][:, ch, :], ident_bf)
                out_sb = gpool.tile([64, 2, 2, 128], F32, tag="out_sb")
                nc.vector.tensor_copy(out_sb, ps_o)
                for c in range(2):
                    nc.sync.dma_start(
                        out_d[c * SC:(c + 1) * SC].rearrange(
                            "s (ch p) -> s ch p", ch=2), out_sb[:, c])

    _split_multi_waits(nc)
    return nc


def _split_multi_waits(nc: bass.Bass):
    """Encode at most ONE semaphore wait per ISA instruction: hoist extras
    onto preceding same-engine NoOp carriers."""
    fn = nc.m.functions[0]
    for blk in fn.blocks:
        insts = list(blk.instructions)
        out = []
        changed = False
        for inst in insts:
            si = inst.sync_info
            waits = list(si.on_wait) if si is not None else []
            if len(waits) > 1:
                changed = True
                for w in waits[:-1]:
                    out.append(mybir.InstNoOp(
                        name=f"I-wsplit-{nc.next_id()}",
                        engine=inst.engine,
                        ins=[], outs=[],
                        sync_info=mybir.SyncInfo(on_wait=[w], on_update=[]),
                    ))
                inst.sync_info = mybir.SyncInfo(
                    on_wait=[waits[-1]], on_update=list(si.on_update))
            out.append(inst)
        if changed:
            blk.instructions = out


_CACHE = {}


def _get_nc(zero_bias: bool) -> bass.Bass:
    if zero_bias not in _CACHE:
        _CACHE[zero_bias] = build(zero_bias)
    return _CACHE[zero_bias]


def _pack_weights(conv_w, attn_w, gru_w, gru_u):
    bf = ml_dtypes.bfloat16
    # z-gate columns negated: sigmoid of the negated preact yields w = 1-z
    gru_w = gru_w.copy(); gru_w[:, :256] *= -1.0
    gru_u = gru_u.copy(); gru_u[:, :256] *= -1.0
    cw = (conv_w[0] if conv_w.ndim == 3 else conv_w).astype(bf)  # [128, 256]
    aw = attn_w.astype(bf).reshape(2, 128, T).transpose(1, 0, 2).reshape(
        128, 2 * T)
    wg = gru_w.astype(bf).reshape(2, 128, 768).transpose(1, 0, 2).reshape(
        128, 1536)
    wu = gru_u.astype(bf).reshape(2, 128, 768).transpose(1, 0, 2).reshape(
        128, 1536)
    ident = np.eye(128, dtype=np.float32).astype(bf)
    return np.ascontiguousarray(
        np.concatenate([cw, aw, wg, wu, ident], axis=1), bf)


def kernel(x, conv_w, conv_b, attn_w, attn_b, gru_w, gru_u, gru_b):
    x = np.asarray(x, dtype=np.float32)
    conv_w = np.asarray(conv_w, dtype=np.float32)
    conv_b = np.asarray(conv_b, dtype=np.float32)
    attn_w = np.asarray(attn_w, dtype=np.float32)
    attn_b = np.asarray(attn_b, dtype=np.float32)
    gru_w = np.asarray(gru_w, dtype=np.float32)
    gru_u = np.asarray(gru_u, dtype=np.float32)
    gru_b = np.asarray(gru_b, dtype=np.float32)

    zero_bias = (
        not conv_b.any() and not attn_b.any() and not gru_b.any())

    nc = _get_nc(zero_bias)

    xs_bf = x.reshape(B * LTMS, T, C_IN).astype(ml_dtypes.bfloat16)
    bfpack = _pack_weights(conv_w, attn_w, gru_w, gru_u)

    in_maps = []
    for c in range(NCORES):
        m = {
            "x_shard": np.ascontiguousarray(xs_bf[c * S: (c + 1) * S]),
            "bfpack": bfpack,
        }
        if not zero_bias:
            bi, br = gru_b[0], gru_b[1]
            comb = bi + br
            gbr = np.zeros((1, 8 * 128), np.float32)
            gbr[0, 0:512] = comb[0:512]          # z0 z1 r0 r1
            gbr[0, 0:256] *= -1.0                # negated z preact -> w
            gbr[0, 512:768] = br[512:768]        # rh0 rh1
            gbr[0, 768:1024] = bi[512:768]       # xh0 xh1
            m["conv_b2"] = np.ascontiguousarray(
                conv_b.reshape(2, 128).T, np.float32)
            m["attn_b"] = attn_b.reshape(1, T).astype(ml_dtypes.bfloat16)
            m["gbias_row"] = gbr.astype(ml_dtypes.bfloat16)
        in_maps.append(m)

    res = run_bass_kernel_spmd(nc, in_maps, core_ids=list(range(NCORES)))
    outs = [res.results[c]["h_out"] for c in range(NCORES)]
    h = np.concatenate(outs, axis=0)  # [1024, 256]
    return h.reshape(B, LTMS, HH).astype(np.float32)


if __name__ == "__main__":
    nc = _get_nc(True)
    print("built ok")


# revision 7
# speedup vs baseline: 1.3241x; 1.0009x over previous
"""Trainium2 Bass kernel for nn_Attention_Encoder (conv1x1 -> time-softmax attention -> relu-GRU).

Sharding: pure data parallelism. 1024 segments split across 8 cores (S=128
per core); weights replicated. v2 redesign vs baseline:

phase A (per segment pair):
  x_T [C,2,T] via transpose-DMA; conv_T = relu(Wc^T x_T) (DVE TSP evac);
  conv_N obtained by PE transposes of the relu'd conv_T (bf16 PSUM
  pass-through, evacuated by a 2x-rate TensorCopy); scores -> exp (ACT,
  fused row-sum); x_att = E*conv on Pool (TT), then *rinv on DVE (4x TSP),
  stored [128, k, S, T] so writes are packed.

phase B: two software-pipelined chains of SC=64 segments (B half a step
  behind A) hide the recurrence latency. Per chain-step: one PSUM tile
  [128, 8, SC] holds z,r,rh,xh (concurrent per-bank accumulation groups,
  sim check skipped -- hardware zeroes only written bytes); ACT evacuates
  xh and runs one combined sigmoid over [z;r]; DVE: t1=r*rh(PSUM),
  q=t1+xh, hh=relu(q) (4x TSP); Pool: d=h-hh, e=z*d, h'=hh+e.
  GRU biases (generic path) are added via rank-1 matmuls into PSUM.
"""

import contextlib
import os
import sys

sys.path.insert(0, "/opt/trn_rl_repo")

import numpy as np
import ml_dtypes

import concourse.bass as bass
import concourse.tile as tile
from concourse import mybir
from concourse.bass_utils import run_bass_kernel_spmd

F32 = mybir.dt.float32
BF16 = mybir.dt.bfloat16
AF = mybir.ActivationFunctionType
OP = mybir.AluOpType

B, LTMS, TTS, C_IN, FF, HH = 64, 16, 256, 128, 256, 256
NCORES = 8
S = (B * LTMS) // NCORES  # 128 segments per core
T = TTS                   # 256 timesteps
SC = S // 2               # 64 segments per chain

# bfpack column layout (bf16): conv_w | attn_w | gru_w | gru_u | identity
BP_CW = 0
BP_AW = BP_CW + FF              # 256
BP_WG = BP_AW + 2 * T           # 768
BP_WU = BP_WG + 2 * 3 * HH      # 2304
BP_ID = BP_WU + 2 * 3 * HH      # 3840
BP_W = BP_ID + 128              # 3968


def build(zero_bias: bool) -> bass.Bass:
    nc = bass.Bass("TRN2", target_bir_lowering=False)

    x_d = nc.dram_tensor("x_shard", [S, T, C_IN], BF16, kind="ExternalInput")
    bp_d = nc.dram_tensor("bfpack", [128, BP_W], BF16, kind="ExternalInput")
    if not zero_bias:
        cb_d = nc.dram_tensor("conv_b2", [128, 2], F32, kind="ExternalInput")
        ab_d = nc.dram_tensor("attn_b", [1, T], BF16, kind="ExternalInput")
        # gru bias rows for rank-1 PSUM adds: [1, 8*128] bf16
        # order: z0 z1 r0 r1 (bi+br) | rh0 rh1 (br_h) | xh0 xh1 (bi_h)
        gb_d = nc.dram_tensor("gbias_row", [1, 8 * 128], BF16,
                              kind="ExternalInput")
    out_d = nc.dram_tensor("h_out", [S, HH], F32, kind="ExternalOutput")

    with tile.TileContext(nc, trace_sim=bool(os.environ.get("KTRACE"))) as tc:
        with contextlib.ExitStack() as ctx:
            singles = ctx.enter_context(tc.tile_pool(name="singles", bufs=1))

            bp_sb = singles.tile([128, BP_W], BF16)
            nc.sync.dma_start(bp_sb, bp_d[:])

            cw_sb = bp_sb[:, BP_CW:BP_CW + FF]
            aw_sb = bp_sb[:, BP_AW:BP_AW + 2 * T].rearrange(
                "p (k n) -> p k n", k=2)
            wg_sb = bp_sb[:, BP_WG:BP_WG + 1536].rearrange(
                "p (k n) -> p k n", k=2)
            wu_sb = bp_sb[:, BP_WU:BP_WU + 1536].rearrange(
                "p (k n) -> p k n", k=2)
            ident_bf = bp_sb[:, BP_ID:BP_ID + 128]

            # global x_att store: [F%128, F-chunk, S, T] bf16 (T packed)
            xatt = singles.tile([128, 2, S, T], BF16)

            if not zero_bias:
                cb_sb = singles.tile([128, 2], F32)
                nc.sync.dma_start(cb_sb, cb_d[:])
                ab_row = singles.tile([1, T], BF16)
                nc.sync.dma_start(ab_row, ab_d[:])
                gb_row = singles.tile([1, 8 * 128], BF16)
                nc.sync.dma_start(gb_row, gb_d[:])
                ones_col = singles.tile([1, 128], BF16)
                nc.vector.memset(ones_col, 1.0)
                ones_sc = ones_col[:, :SC]

            # ---------------- phase A ----------------
            apool = ctx.enter_context(tc.tile_pool(name="apool", bufs=4))
            with contextlib.ExitStack() as actx:
                apsum = actx.enter_context(
                    tc.tile_pool(name="apsum", bufs=1, space="PSUM"))

                # PE warmup: consume the weight-pack DMA on PE early
                # (borrows the ps_cn tag so phase A stays within 8 PSUM banks)
                ps_w1 = apsum.tile([128, 128], BF16, tag="ps_cn", bufs=2)
                nc.tensor.transpose(ps_w1, ident_bf, ident_bf)

                for s2 in range(S // 2):
                    s = 2 * s2
                    x_t = apool.tile([128, 2, T], BF16, tag="x_t", bufs=6)
                    nc.sync.dma_start_transpose(x_t[:, 0, :], x_d[s])
                    nc.sync.dma_start_transpose(x_t[:, 1, :], x_d[s + 1])

                    # conv_T = relu(W_c^T @ x_T): [F(2ch), seg, T]
                    ps_ct = apsum.tile([128, 2, 2, T], F32, tag="ps_ct",
                                       bufs=1)
                    for m in range(2):
                        nc.tensor.matmul(
                            ps_ct[:, m, :, :], cw_sb[:, bass.ts(m, 128)],
                            x_t, start=True, stop=True)
                    conv_t = apool.tile([128, 2, 2, T], BF16, tag="conv_t")
                    if zero_bias:
                        # single wide evac+relu amortizes the PSUM access
                        nc.vector.tensor_scalar_max(conv_t, ps_ct, 0.0)
                    else:
                        for mc in range(2):
                            nc.vector.tensor_scalar(
                                conv_t[:, mc, :, :], ps_ct[:, mc, :, :],
                                cb_sb[:, mc:mc + 1], 0.0, OP.add, OP.max)

                    # conv_N via PE transposes of relu'd conv_T (bf16 psum)
                    ps_cn = apsum.tile([128, 2, 2, FF], BF16, tag="ps_cn",
                                       bufs=2)
                    for seg in range(2):
                        for tch in range(2):
                            for m in range(2):
                                nc.tensor.transpose(
                                    ps_cn[:, seg, tch, bass.ts(m, 128)],
                                    conv_t[:, m, seg, bass.ts(tch, 128)],
                                    ident_bf)
                    conv_n = apool.tile([128, 2, 2, FF], BF16, tag="conv_n")
                    nc.vector.tensor_copy(conv_n, ps_cn)

                    # scores = conv_N^T @ A (+ b): [seg, F-ch, T]
                    ps_s = apsum.tile([128, 2, 2, T], F32, tag="ps_s", bufs=2)
                    for seg in range(2):
                        for m in range(2):
                            for k in range(2):
                                nc.tensor.matmul(
                                    ps_s[:, seg, m, :],
                                    conv_n[:, seg, k, bass.ts(m, 128)],
                                    aw_sb[:, k, :],
                                    start=(k == 0),
                                    stop=(k == 1) and zero_bias)
                            if not zero_bias:
                                nc.tensor.matmul(
                                    ps_s[:, seg, m, :], ones_col, ab_row,
                                    start=False, stop=True)

                    ee = apool.tile([128, 2, 2, T], BF16, tag="ee")
                    esum = apool.tile([128, 4], F32, tag="esum")
                    es4 = esum.rearrange("p (a b) -> p a b", a=2)
                    for seg in range(2):
                        for m in range(2):
                            nc.scalar.activation(
                                ee[:, seg, m, :], ps_s[:, seg, m, :], AF.Exp,
                                accum_out=es4[:, seg, m:m + 1])
                    rinv = apool.tile([128, 4], F32, tag="rinv")
                    nc.vector.reciprocal(rinv, esum)
                    ri4 = rinv.rearrange("p (a b) -> p a b", a=2)

                    # x_att[:, m, s+seg, :] = E * rinv * conv_T  (packed T)
                    # split: ec = E*conv on Pool (TT), then *rinv on DVE (4x)
                    ec = apool.tile([128, 2, 2, T], BF16, tag="ec")
                    for seg in range(2):
                        nc.gpsimd.tensor_mul(
                            ec[:, seg, :, :], ee[:, seg, :, :],
                            conv_t[:, :, seg, :])
                    for seg in range(2):
                        for m in range(2):
                            nc.vector.tensor_scalar_mul(
                                xatt[:, m, s + seg, :], ec[:, seg, m, :],
                                ri4[:, seg, m:m + 1])

            # ---------------- phase B: GRU over T steps, 2 chains ----------
            # gate columns in W/U: z=[0,256) m0,1 ; r=[256,512) m2,3 ;
            # h=[512,768) m4,5
            # psum tile layout [128, 8, SC]: z0 z1 r0 r1 | rh0 rh1 | xh0 xh1
            with contextlib.ExitStack() as bctx:
                hpool = bctx.enter_context(tc.tile_pool(name="hpool", bufs=2))
                gpool = bctx.enter_context(tc.tile_pool(name="gpool", bufs=3))
                bpsum = bctx.enter_context(
                    tc.tile_pool(name="bpsum", bufs=1, space="PSUM"))

                h_prev = [None, None]
                pend = [None, None]  # (t, ps, rz, xh_sb) awaiting elementwise

                def emit_pe_act(c, t):
                    """Matmuls + sigmoid + xh evac for (chain c, step t)."""
                    cb = c * SC
                    ps = bpsum.tile([128, 8, SC], F32, tag=f"ps{c}", bufs=3,
                                    name=f"ps{c}")
                    hp = h_prev[c]

                    # x-part matmuls (independent of h)
                    zr_stop = (t == 0) and zero_bias
                    for j, m in enumerate((0, 1)):      # z gates
                        for k in range(2):
                            nc.tensor.matmul(
                                ps[:, j, :], wg_sb[:, k, bass.ts(m, 128)],
                                xatt[:, k, cb:cb + SC, t],
                                start=(k == 0), stop=(k == 1) and zr_stop,
                                skip_group_check=True)
                    for j, m in enumerate((2, 3)):      # r gates
                        for k in range(2):
                            nc.tensor.matmul(
                                ps[:, 2 + j, :],
                                wg_sb[:, k, bass.ts(m, 128)],
                                xatt[:, k, cb:cb + SC, t],
                                start=(k == 0), stop=(k == 1) and zr_stop,
                                skip_group_check=True)
                    for j, m in enumerate((4, 5)):      # h gate (xh)
                        for k in range(2):
                            nc.tensor.matmul(
                                ps[:, 6 + j, :],
                                wg_sb[:, k, bass.ts(m, 128)],
                                xatt[:, k, cb:cb + SC, t],
                                start=(k == 0),
                                stop=(k == 1) and zero_bias,
                                skip_group_check=True)

                    if not zero_bias:
                        # rank-1 bias adds; z0..r1 into [0:4],
                        # xh into [6:8], rh (br_h) into [4:6]
                        for j in range(4):
                            nc.tensor.matmul(
                                ps[:, j, :], gb_row[:, bass.ts(j, 128)],
                                ones_sc, start=False, stop=(t == 0),
                                skip_group_check=True)
                        for j in range(2):
                            nc.tensor.matmul(
                                ps[:, 6 + j, :],
                                gb_row[:, bass.ts(6 + j, 128)],
                                ones_sc, start=False, stop=True,
                                skip_group_check=True)
                        for j in range(2):
                            nc.tensor.matmul(
                                ps[:, 4 + j, :],
                                gb_row[:, bass.ts(4 + j, 128)],
                                ones_sc, start=True, stop=(t == 0),
                                skip_group_check=True)

                    # ACT evacuates xh early (depends only on Wx)
                    xh_sb = gpool.tile([128, 2, SC], BF16, tag=f"xh{c}",
                                       bufs=2, name=f"xh{c}")
                    nc.scalar.copy(xh_sb, ps[:, 6:8, :])

                    rz = gpool.tile([128, 4, SC], BF16, tag=f"rz{c}",
                                    bufs=2, name=f"rz{c}")
                    if t > 0:
                        # U-part: r,z first (gate the sigmoid), then rh
                        for j, m in enumerate((2, 3)):
                            for k in range(2):
                                nc.tensor.matmul(
                                    ps[:, 2 + j, :],
                                    wu_sb[:, k, bass.ts(m, 128)],
                                    hp[:, k, :],
                                    start=False, stop=(k == 1),
                                    skip_group_check=True)
                        for j, m in enumerate((0, 1)):
                            for k in range(2):
                                nc.tensor.matmul(
                                    ps[:, j, :],
                                    wu_sb[:, k, bass.ts(m, 128)],
                                    hp[:, k, :],
                                    start=False, stop=(k == 1),
                                    skip_group_check=True)
                        for j, m in enumerate((4, 5)):  # rh
                            for k in range(2):
                                nc.tensor.matmul(
                                    ps[:, 4 + j, :],
                                    wu_sb[:, k, bass.ts(m, 128)],
                                    hp[:, k, :],
                                    start=(k == 0) and zero_bias,
                                    stop=(k == 1),
                                    skip_group_check=True)
                    # sigmoid over [z;r] in one ACT op
                    nc.scalar.activation(rz, ps[:, 0:4, :], AF.Sigmoid)
                    pend[c] = (t, ps, rz, xh_sb)

                def emit_dve(c):
                    """Elementwise chain for the pending (chain c) step.
                    The d/e blend ops run on Pool to cut DVE occupancy."""
                    t, ps, rz, xh_sb = pend[c]
                    hp = h_prev[c]
                    h_new = hpool.tile([128, 2, SC], BF16, tag=f"h{c}",
                                       name=f"h{c}")
                    hh = gpool.tile([128, 2, SC], BF16, tag=f"hh{c}",
                                    bufs=2, name=f"hh{c}")
                    d = gpool.tile([128, 2, SC], BF16, tag=f"d{c}",
                                   bufs=2, name=f"d{c}")
                    e = gpool.tile([128, 2, SC], BF16, tag=f"e{c}",
                                   bufs=2, name=f"e{c}")
                    have_rh = (t > 0) or not zero_bias
                    # rz[:,0:2] is already w = 1-z (z weights negated on
                    # host). Off-chain on Pool: m1 = z*h = h - w*h
                    w = rz[:, 0:2, :]
                    if t > 0:
                        mw = gpool.tile([128, 2, SC], BF16, tag=f"mw{c}",
                                        bufs=2, name=f"mw{c}")
                        m1 = gpool.tile([128, 2, SC], BF16, tag=f"m1{c}",
                                        bufs=2, name=f"m1{c}")
                        nc.gpsimd.tensor_mul(mw, w, hp)
                        nc.gpsimd.tensor_sub(m1, hp, mw)
                    if have_rh:
                        t1 = gpool.tile([128, 2, SC], BF16, tag=f"t1{c}",
                                        bufs=2, name=f"t1{c}")
                        q = gpool.tile([128, 2, SC], BF16, tag=f"q{c}",
                                       bufs=2, name=f"q{c}")
                        nc.vector.tensor_mul(t1, rz[:, 2:4, :], ps[:, 4:6, :])
                        nc.vector.tensor_add(q, t1, xh_sb)
                    else:
                        q = xh_sb
                    # hh = relu(q) (TSP, 4x); then only 2 Pool ops on-chain:
                    # h' = w*hh + m1
                    nc.vector.tensor_scalar_max(hh, q, 0.0)
                    if t > 0:
                        nc.gpsimd.tensor_mul(e, w, hh)         # e = (1-z)*hh
                        nc.gpsimd.tensor_add(h_new, e, m1)
                    else:
                        nc.gpsimd.tensor_mul(h_new, w, hh)     # h0 = 0
                    h_prev[c] = h_new

                # software-pipelined: chain B runs half a step behind A.
                # Absolute-time pins phase-lock the two chains: a pin that's
                # already past is a no-op, so transient overruns self-correct.
                pin_base = float(os.environ.get("PIN_BASE", "228000"))  # inert unless PIN_P>0
                pin_p = float(os.environ.get("PIN_P", "0"))
                for t in range(T):
                    tp = pin_base + t * pin_p
                    with tc.tile_wait_until(tp / 1e6, enable=pin_p > 0):
                        emit_pe_act(0, t)
                    if t > 0:
                        with tc.tile_wait_until(tp / 1e6,
                                                enable=pin_p > 0):
                            emit_dve(1)
                    with tc.tile_wait_until((tp + 0.5 * pin_p) / 1e6,
                                            enable=pin_p > 0):
                        emit_pe_act(1, t)
                    with tc.tile_wait_until((tp + 0.46 * pin_p) / 1e6,
                                            enable=pin_p > 0):
                        emit_dve(0)
                emit_dve(1)

                # output: transpose h back to [S, H] and store fp32
                ps_o = bpsum.tile([64, 2, 2, 128], BF16, tag="ps_o", bufs=1)
                for c in range(2):
                    for ch in range(2):
                        nc.tensor.transpose(
                            ps_o[:, c, ch, :], h_prev[c][:, ch, :], ident_bf)
                out_sb = gpool.tile([64, 2, 2, 128], F32, tag="out_sb")
                nc.vector.tensor_copy(out_sb, ps_o)
                for c in range(2):
                    nc.sync.dma_start(
                        out_d[c * SC:(c + 1) * SC].rearrange(
                            "s (ch p) -> s ch p", ch=2), out_sb[:, c])

    _split_multi_waits(nc)
    return nc


def _split_multi_waits(nc: bass.Bass):
    """Encode at most ONE semaphore wait per ISA instruction: hoist extras
    onto preceding same-engine NoOp carriers."""
    fn = nc.m.functions[0]
    for blk in fn.blocks:
        insts = list(blk.instructions)
        out = []
        changed = False
        for inst in insts:
            si = inst.sync_info
            waits = list(si.on_wait) if si is not None else []
            if len(waits) > 1:
                changed = True
                for w in waits[:-1]:
                    out.append(mybir.InstNoOp(
                        name=f"I-wsplit-{nc.next_id()}",
                        engine=inst.engine,
                        ins=[], outs=[],
                        sync_info=mybir.SyncInfo(on_wait=[w], on_update=[]),
                    ))
                inst.sync_info = mybir.SyncInfo(
                    on_wait=[waits[-1]], on_update=list(si.on_update))
            out.append(inst)
        if changed:
            blk.instructions = out


_CACHE = {}


def _get_nc(zero_bias: bool) -> bass.Bass:
    if zero_bias not in _CACHE:
        _CACHE[zero_bias] = build(zero_bias)
    return _CACHE[zero_bias]


def _pack_weights(conv_w, attn_w, gru_w, gru_u):
    bf = ml_dtypes.bfloat16
    # z-gate columns negated: sigmoid of the negated preact yields w = 1-z
    gru_w = gru_w.copy(); gru_w[:, :256] *= -1.0
    gru_u = gru_u.copy(); gru_u[:, :256] *= -1.0
    cw = (conv_w[0] if conv_w.ndim == 3 else conv_w).astype(bf)  # [128, 256]
    aw = attn_w.astype(bf).reshape(2, 128, T).transpose(1, 0, 2).reshape(
        128, 2 * T)
    wg = gru_w.astype(bf).reshape(2, 128, 768).transpose(1, 0, 2).reshape(
        128, 1536)
    wu = gru_u.astype(bf).reshape(2, 128, 768).transpose(1, 0, 2).reshape(
        128, 1536)
    ident = np.eye(128, dtype=np.float32).astype(bf)
    return np.ascontiguousarray(
        np.concatenate([cw, aw, wg, wu, ident], axis=1), bf)


def kernel(x, conv_w, conv_b, attn_w, attn_b, gru_w, gru_u, gru_b):
    x = np.asarray(x, dtype=np.float32)
    conv_w = np.asarray(conv_w, dtype=np.float32)
    conv_b = np.asarray(conv_b, dtype=np.float32)
    attn_w = np.asarray(attn_w, dtype=np.float32)
    attn_b = np.asarray(attn_b, dtype=np.float32)
    gru_w = np.asarray(gru_w, dtype=np.float32)
    gru_u = np.asarray(gru_u, dtype=np.float32)
    gru_b = np.asarray(gru_b, dtype=np.float32)

    zero_bias = (
        not conv_b.any() and not attn_b.any() and not gru_b.any())

    nc = _get_nc(zero_bias)

    xs_bf = x.reshape(B * LTMS, T, C_IN).astype(ml_dtypes.bfloat16)
    bfpack = _pack_weights(conv_w, attn_w, gru_w, gru_u)

    in_maps = []
    for c in range(NCORES):
        m = {
            "x_shard": np.ascontiguousarray(xs_bf[c * S: (c + 1) * S]),
            "bfpack": bfpack,
        }
        if not zero_bias:
            bi, br = gru_b[0], gru_b[1]
            comb = bi + br
            gbr = np.zeros((1, 8 * 128), np.float32)
            gbr[0, 0:512] = comb[0:512]          # z0 z1 r0 r1
            gbr[0, 0:256] *= -1.0                # negated z preact -> w
            gbr[0, 512:768] = br[512:768]        # rh0 rh1
            gbr[0, 768:1024] = bi[512:768]       # xh0 xh1
            m["conv_b2"] = np.ascontiguousarray(
                conv_b.reshape(2, 128).T, np.float32)
            m["attn_b"] = attn_b.reshape(1, T).astype(ml_dtypes.bfloat16)
            m["gbias_row"] = gbr.astype(ml_dtypes.bfloat16)
        in_maps.append(m)

    res = run_bass_kernel_spmd(nc, in_maps, core_ids=list(range(NCORES)))
    outs = [res.results[c]["h_out"] for c in range(NCORES)]
    h = np.concatenate(outs, axis=0)  # [1024, 256]
    return h.reshape(B, LTMS, HH).astype(np.float32)


if __name__ == "__main__":
    nc = _get_nc(True)
    print("built ok")


# revision 8
# speedup vs baseline: 1.3355x; 1.0086x over previous
"""Trainium2 Bass kernel for nn_Attention_Encoder (conv1x1 -> time-softmax attention -> relu-GRU).

Sharding: pure data parallelism. 1024 segments split across 8 cores (S=128
per core); weights replicated. v2 redesign vs baseline:

phase A (per segment pair):
  x_T [C,2,T] via transpose-DMA; conv_T = relu(Wc^T x_T) (DVE TSP evac);
  conv_N obtained by PE transposes of the relu'd conv_T (bf16 PSUM
  pass-through, evacuated by a 2x-rate TensorCopy); scores -> exp (ACT,
  fused row-sum); x_att = E*conv on Pool (TT), then *rinv on DVE (4x TSP),
  stored [128, k, S, T] so writes are packed.

phase B: two software-pipelined chains of SC=64 segments (B half a step
  behind A) hide the recurrence latency. Per chain-step: one PSUM tile
  [128, 8, SC] holds z,r,rh,xh (concurrent per-bank accumulation groups,
  sim check skipped -- hardware zeroes only written bytes); ACT evacuates
  xh and runs one combined sigmoid over [z;r]; DVE: t1=r*rh(PSUM),
  q=t1+xh, hh=relu(q) (4x TSP); Pool: d=h-hh, e=z*d, h'=hh+e.
  GRU biases (generic path) are added via rank-1 matmuls into PSUM.
"""

import contextlib
import os
import sys

sys.path.insert(0, "/opt/trn_rl_repo")

import numpy as np
import ml_dtypes

import concourse.bass as bass
import concourse.tile as tile
from concourse import mybir
from concourse.bass_utils import run_bass_kernel_spmd

F32 = mybir.dt.float32
BF16 = mybir.dt.bfloat16
AF = mybir.ActivationFunctionType
OP = mybir.AluOpType

B, LTMS, TTS, C_IN, FF, HH = 64, 16, 256, 128, 256, 256
NCORES = 8
S = (B * LTMS) // NCORES  # 128 segments per core
T = TTS                   # 256 timesteps
SC = S // 2               # 64 segments per chain

# bfpack column layout (bf16): conv_w | attn_w | gru_w | gru_u | identity
BP_CW = 0
BP_AW = BP_CW + FF              # 256
BP_WG = BP_AW + 2 * T           # 768
BP_WU = BP_WG + 2 * 3 * HH      # 2304
BP_ID = BP_WU + 2 * 3 * HH      # 3840
BP_W = BP_ID + 128              # 3968


def build(zero_bias: bool) -> bass.Bass:
    nc = bass.Bass("TRN2", target_bir_lowering=False)

    x_d = nc.dram_tensor("x_shard", [S, T, C_IN], BF16, kind="ExternalInput")
    bp_d = nc.dram_tensor("bfpack", [128, BP_W], BF16, kind="ExternalInput")
    if not zero_bias:
        cb_d = nc.dram_tensor("conv_b2", [128, 2], F32, kind="ExternalInput")
        ab_d = nc.dram_tensor("attn_b", [1, T], BF16, kind="ExternalInput")
        # gru bias rows for rank-1 PSUM adds: [1, 8*128] bf16
        # order: z0 z1 r0 r1 (bi+br) | rh0 rh1 (br_h) | xh0 xh1 (bi_h)
        gb_d = nc.dram_tensor("gbias_row", [1, 8 * 128], BF16,
                              kind="ExternalInput")
    out_d = nc.dram_tensor("h_out", [S, HH], F32, kind="ExternalOutput")

    with tile.TileContext(nc, trace_sim=bool(os.environ.get("KTRACE"))) as tc:
        with contextlib.ExitStack() as ctx:
            singles = ctx.enter_context(tc.tile_pool(name="singles", bufs=1))

            bp_sb = singles.tile([128, BP_W], BF16)
            nc.sync.dma_start(bp_sb, bp_d[:])

            cw_sb = bp_sb[:, BP_CW:BP_CW + FF]
            aw_sb = bp_sb[:, BP_AW:BP_AW + 2 * T].rearrange(
                "p (k n) -> p k n", k=2)
            wg_sb = bp_sb[:, BP_WG:BP_WG + 1536].rearrange(
                "p (k n) -> p k n", k=2)
            wu_sb = bp_sb[:, BP_WU:BP_WU + 1536].rearrange(
                "p (k n) -> p k n", k=2)
            ident_bf = bp_sb[:, BP_ID:BP_ID + 128]

            # global x_att store: [F%128, F-chunk, S, T] bf16 (T packed)
            xatt = singles.tile([128, 2, S, T], BF16)

            if not zero_bias:
                cb_sb = singles.tile([128, 2], F32)
                nc.sync.dma_start(cb_sb, cb_d[:])
                ab_row = singles.tile([1, T], BF16)
                nc.sync.dma_start(ab_row, ab_d[:])
                gb_row = singles.tile([1, 8 * 128], BF16)
                nc.sync.dma_start(gb_row, gb_d[:])
                ones_col = singles.tile([1, 128], BF16)
                nc.vector.memset(ones_col, 1.0)
                ones_sc = ones_col[:, :SC]

            # ---------------- phase A ----------------
            apool = ctx.enter_context(tc.tile_pool(name="apool", bufs=4))
            with contextlib.ExitStack() as actx:
                apsum = actx.enter_context(
                    tc.tile_pool(name="apsum", bufs=1, space="PSUM"))

                # PE warmup: consume the weight-pack DMA on PE early
                # (borrows the ps_cn tag so phase A stays within 8 PSUM banks)
                ps_w1 = apsum.tile([128, 128], BF16, tag="ps_cn", bufs=2)
                nc.tensor.transpose(ps_w1, ident_bf, ident_bf)

                for s2 in range(S // 2):
                    s = 2 * s2
                    x_t = apool.tile([128, 2, T], BF16, tag="x_t", bufs=6)
                    nc.sync.dma_start_transpose(x_t[:, 0, :], x_d[s])
                    nc.sync.dma_start_transpose(x_t[:, 1, :], x_d[s + 1])

                    # conv_T = relu(W_c^T @ x_T): [F(2ch), seg, T]
                    ps_ct = apsum.tile([128, 2, 2, T], F32, tag="ps_ct",
                                       bufs=1)
                    for m in range(2):
                        nc.tensor.matmul(
                            ps_ct[:, m, :, :], cw_sb[:, bass.ts(m, 128)],
                            x_t, start=True, stop=True)
                    conv_t = apool.tile([128, 2, 2, T], BF16, tag="conv_t")
                    if zero_bias:
                        # single wide evac+relu amortizes the PSUM access
                        nc.vector.tensor_scalar_max(conv_t, ps_ct, 0.0)
                    else:
                        for mc in range(2):
                            nc.vector.tensor_scalar(
                                conv_t[:, mc, :, :], ps_ct[:, mc, :, :],
                                cb_sb[:, mc:mc + 1], 0.0, OP.add, OP.max)

                    # conv_N via PE transposes of relu'd conv_T (bf16 psum)
                    ps_cn = apsum.tile([128, 2, 2, FF], BF16, tag="ps_cn",
                                       bufs=2)
                    for seg in range(2):
                        for tch in range(2):
                            for m in range(2):
                                nc.tensor.transpose(
                                    ps_cn[:, seg, tch, bass.ts(m, 128)],
                                    conv_t[:, m, seg, bass.ts(tch, 128)],
                                    ident_bf)
                    conv_n = apool.tile([128, 2, 2, FF], BF16, tag="conv_n")
                    nc.vector.tensor_copy(conv_n, ps_cn)

                    # scores = conv_N^T @ A (+ b): [seg, F-ch, T]
                    ps_s = apsum.tile([128, 2, 2, T], F32, tag="ps_s", bufs=2)
                    for seg in range(2):
                        for m in range(2):
                            for k in range(2):
                                nc.tensor.matmul(
                                    ps_s[:, seg, m, :],
                                    conv_n[:, seg, k, bass.ts(m, 128)],
                                    aw_sb[:, k, :],
                                    start=(k == 0),
                                    stop=(k == 1) and zero_bias)
                            if not zero_bias:
                                nc.tensor.matmul(
                                    ps_s[:, seg, m, :], ones_col, ab_row,
                                    start=False, stop=True)

                    ee = apool.tile([128, 2, 2, T], BF16, tag="ee")
                    esum = apool.tile([128, 4], F32, tag="esum")
                    es4 = esum.rearrange("p (a b) -> p a b", a=2)
                    for seg in range(2):
                        for m in range(2):
                            nc.scalar.activation(
                                ee[:, seg, m, :], ps_s[:, seg, m, :], AF.Exp,
                                accum_out=es4[:, seg, m:m + 1])
                    rinv = apool.tile([128, 4], F32, tag="rinv")
                    nc.vector.reciprocal(rinv, esum)
                    ri4 = rinv.rearrange("p (a b) -> p a b", a=2)

                    # x_att[:, m, s+seg, :] = E * rinv * conv_T  (packed T)
                    # split: ec = E*conv on Pool (TT), then *rinv on DVE (4x)
                    ec = apool.tile([128, 2, 2, T], BF16, tag="ec")
                    for seg in range(2):
                        nc.gpsimd.tensor_mul(
                            ec[:, seg, :, :], ee[:, seg, :, :],
                            conv_t[:, :, seg, :])
                    for seg in range(2):
                        for m in range(2):
                            nc.gpsimd.tensor_mul(
                                xatt[:, m, s + seg, :], ec[:, seg, m, :],
                                ri4[:, seg, m:m + 1].broadcast_to([128, T]))

            # ---------------- phase B: GRU over T steps, 2 chains ----------
            # gate columns in W/U: z=[0,256) m0,1 ; r=[256,512) m2,3 ;
            # h=[512,768) m4,5
            # psum tile layout [128, 8, SC]: z0 z1 r0 r1 | rh0 rh1 | xh0 xh1
            with contextlib.ExitStack() as bctx:
                hpool = bctx.enter_context(tc.tile_pool(name="hpool", bufs=2))
                gpool = bctx.enter_context(tc.tile_pool(name="gpool", bufs=3))
                bpsum = bctx.enter_context(
                    tc.tile_pool(name="bpsum", bufs=1, space="PSUM"))

                h_prev = [None, None]
                pend = [None, None]  # (t, ps, rz, xh_sb) awaiting elementwise

                def emit_pe_act(c, t):
                    """Matmuls + sigmoid + xh evac for (chain c, step t)."""
                    cb = c * SC
                    ps = bpsum.tile([128, 8, SC], F32, tag=f"ps{c}", bufs=3,
                                    name=f"ps{c}")
                    hp = h_prev[c]

                    # x-part matmuls (independent of h)
                    zr_stop = (t == 0) and zero_bias
                    for j, m in enumerate((0, 1)):      # z gates
                        for k in range(2):
                            nc.tensor.matmul(
                                ps[:, j, :], wg_sb[:, k, bass.ts(m, 128)],
                                xatt[:, k, cb:cb + SC, t],
                                start=(k == 0), stop=(k == 1) and zr_stop,
                                skip_group_check=True)
                    for j, m in enumerate((2, 3)):      # r gates
                        for k in range(2):
                            nc.tensor.matmul(
                                ps[:, 2 + j, :],
                                wg_sb[:, k, bass.ts(m, 128)],
                                xatt[:, k, cb:cb + SC, t],
                                start=(k == 0), stop=(k == 1) and zr_stop,
                                skip_group_check=True)
                    for j, m in enumerate((4, 5)):      # h gate (xh)
                        for k in range(2):
                            nc.tensor.matmul(
                                ps[:, 6 + j, :],
                                wg_sb[:, k, bass.ts(m, 128)],
                                xatt[:, k, cb:cb + SC, t],
                                start=(k == 0),
                                stop=(k == 1) and zero_bias,
                                skip_group_check=True)

                    if not zero_bias:
                        # rank-1 bias adds; z0..r1 into [0:4],
                        # xh into [6:8], rh (br_h) into [4:6]
                        for j in range(4):
                            nc.tensor.matmul(
                                ps[:, j, :], gb_row[:, bass.ts(j, 128)],
                                ones_sc, start=False, stop=(t == 0),
                                skip_group_check=True)
                        for j in range(2):
                            nc.tensor.matmul(
                                ps[:, 6 + j, :],
                                gb_row[:, bass.ts(6 + j, 128)],
                                ones_sc, start=False, stop=True,
                                skip_group_check=True)
                        for j in range(2):
                            nc.tensor.matmul(
                                ps[:, 4 + j, :],
                                gb_row[:, bass.ts(4 + j, 128)],
                                ones_sc, start=True, stop=(t == 0),
                                skip_group_check=True)

                    # ACT evacuates xh early (depends only on Wx)
                    xh_sb = gpool.tile([128, 2, SC], BF16, tag=f"xh{c}",
                                       bufs=2, name=f"xh{c}")
                    nc.scalar.copy(xh_sb, ps[:, 6:8, :])

                    rz = gpool.tile([128, 4, SC], BF16, tag=f"rz{c}",
                                    bufs=2, name=f"rz{c}")
                    if t > 0:
                        # U-part: r,z first (gate the sigmoid), then rh
                        for j, m in enumerate((2, 3)):
                            for k in range(2):
                                nc.tensor.matmul(
                                    ps[:, 2 + j, :],
                                    wu_sb[:, k, bass.ts(m, 128)],
                                    hp[:, k, :],
                                    start=False, stop=(k == 1),
                                    skip_group_check=True)
                        for j, m in enumerate((0, 1)):
                            for k in range(2):
                                nc.tensor.matmul(
                                    ps[:, j, :],
                                    wu_sb[:, k, bass.ts(m, 128)],
                                    hp[:, k, :],
                                    start=False, stop=(k == 1),
                                    skip_group_check=True)
                        for j, m in enumerate((4, 5)):  # rh
                            for k in range(2):
                                nc.tensor.matmul(
                                    ps[:, 4 + j, :],
                                    wu_sb[:, k, bass.ts(m, 128)],
                                    hp[:, k, :],
                                    start=(k == 0) and zero_bias,
                                    stop=(k == 1),
                                    skip_group_check=True)
                    # sigmoid over [z;r] in one ACT op
                    nc.scalar.activation(rz, ps[:, 0:4, :], AF.Sigmoid)
                    pend[c] = (t, ps, rz, xh_sb)

                def emit_dve(c):
                    """Elementwise chain for the pending (chain c) step.
                    The d/e blend ops run on Pool to cut DVE occupancy."""
                    t, ps, rz, xh_sb = pend[c]
                    hp = h_prev[c]
                    h_new = hpool.tile([128, 2, SC], BF16, tag=f"h{c}",
                                       name=f"h{c}")
                    hh = gpool.tile([128, 2, SC], BF16, tag=f"hh{c}",
                                    bufs=2, name=f"hh{c}")
                    d = gpool.tile([128, 2, SC], BF16, tag=f"d{c}",
                                   bufs=2, name=f"d{c}")
                    e = gpool.tile([128, 2, SC], BF16, tag=f"e{c}",
                                   bufs=2, name=f"e{c}")
                    have_rh = (t > 0) or not zero_bias
                    # rz[:,0:2] is already w = 1-z (z weights negated on
                    # host). Off-chain on Pool: m1 = z*h = h - w*h
                    w = rz[:, 0:2, :]
                    if t > 0:
                        mw = gpool.tile([128, 2, SC], BF16, tag=f"mw{c}",
                                        bufs=2, name=f"mw{c}")
                        m1 = gpool.tile([128, 2, SC], BF16, tag=f"m1{c}",
                                        bufs=2, name=f"m1{c}")
                        nc.gpsimd.tensor_mul(mw, w, hp)
                        nc.gpsimd.tensor_sub(m1, hp, mw)
                    if have_rh:
                        t1 = gpool.tile([128, 2, SC], BF16, tag=f"t1{c}",
                                        bufs=2, name=f"t1{c}")
                        q = gpool.tile([128, 2, SC], BF16, tag=f"q{c}",
                                       bufs=2, name=f"q{c}")
                        nc.vector.tensor_mul(t1, rz[:, 2:4, :], ps[:, 4:6, :])
                        nc.vector.tensor_add(q, t1, xh_sb)
                    else:
                        q = xh_sb
                    # hh = relu(q) (TSP, 4x); then only 2 Pool ops on-chain:
                    # h' = w*hh + m1
                    nc.vector.tensor_scalar_max(hh, q, 0.0)
                    if t > 0:
                        nc.gpsimd.tensor_mul(e, w, hh)         # e = (1-z)*hh
                        nc.gpsimd.tensor_add(h_new, e, m1)
                    else:
                        nc.gpsimd.tensor_mul(h_new, w, hh)     # h0 = 0
                    h_prev[c] = h_new

                # software-pipelined: chain B runs half a step behind A.
                # Absolute-time pins phase-lock the two chains: a pin that's
                # already past is a no-op, so transient overruns self-correct.
                pin_base = float(os.environ.get("PIN_BASE", "228000"))  # inert unless PIN_P>0
                pin_p = float(os.environ.get("PIN_P", "0"))
                for t in range(T):
                    tp = pin_base + t * pin_p
                    with tc.tile_wait_until(tp / 1e6, enable=pin_p > 0):
                        emit_pe_act(0, t)
                    if t > 0:
                        with tc.tile_wait_until(tp / 1e6,
                                                enable=pin_p > 0):
                            emit_dve(1)
                    with tc.tile_wait_until((tp + 0.5 * pin_p) / 1e6,
                                            enable=pin_p > 0):
                        emit_pe_act(1, t)
                    with tc.tile_wait_until((tp + 0.46 * pin_p) / 1e6,
                                            enable=pin_p > 0):
                        emit_dve(0)
                emit_dve(1)

                # output: transpose h back to [S, H] and store fp32
                ps_o = bpsum.tile([64, 2, 2, 128], BF16, tag="ps_o", bufs=1)
                for c in range(2):
                    for ch in range(2):
                        nc.tensor.transpose(
                            ps_o[:, c, ch, :], h_prev[c][:, ch, :], ident_bf)
                out_sb = gpool.tile([64, 2, 2, 128], F32, tag="out_sb")
                nc.vector.tensor_copy(out_sb, ps_o)
                for c in range(2):
                    nc.sync.dma_start(
                        out_d[c * SC:(c + 1) * SC].rearrange(
                            "s (ch p) -> s ch p", ch=2), out_sb[:, c])

    _split_multi_waits(nc)
    return nc


def _split_multi_waits(nc: bass.Bass):
    """Encode at most ONE semaphore wait per ISA instruction: hoist extras
    onto preceding same-engine NoOp carriers."""
    fn = nc.m.functions[0]
    for blk in fn.blocks:
        insts = list(blk.instructions)
        out = []
        changed = False
        for inst in insts:
            si = inst.sync_info
            waits = list(si.on_wait) if si is not None else []
            if len(waits) > 1:
                changed = True
                for w in waits[:-1]:
                    out.append(mybir.InstNoOp(
                        name=f"I-wsplit-{nc.next_id()}",
                        engine=inst.engine,
                        ins=[], outs=[],
                        sync_info=mybir.SyncInfo(on_wait=[w], on_update=[]),
                    ))
                inst.sync_info = mybir.SyncInfo(
                    on_wait=[waits[-1]], on_update=list(si.on_update))
            out.append(inst)
        if changed:
            blk.instructions = out


_CACHE = {}


def _get_nc(zero_bias: bool) -> bass.Bass:
    if zero_bias not in _CACHE:
        _CACHE[zero_bias] = build(zero_bias)
    return _CACHE[zero_bias]


def _pack_weights(conv_w, attn_w, gru_w, gru_u):
    bf = ml_dtypes.bfloat16
    # z-gate columns negated: sigmoid of the negated preact yields w = 1-z
    gru_w = gru_w.copy(); gru_w[:, :256] *= -1.0
    gru_u = gru_u.copy(); gru_u[:, :256] *= -1.0
    cw = (conv_w[0] if conv_w.ndim == 3 else conv_w).astype(bf)  # [128, 256]
    aw = attn_w.astype(bf).reshape(2, 128, T).transpose(1, 0, 2).reshape(
        128, 2 * T)
    wg = gru_w.astype(bf).reshape(2, 128, 768).transpose(1, 0, 2).reshape(
        128, 1536)
    wu = gru_u.astype(bf).reshape(2, 128, 768).transpose(1, 0, 2).reshape(
        128, 1536)
    ident = np.eye(128, dtype=np.float32).astype(bf)
    return np.ascontiguousarray(
        np.concatenate([cw, aw, wg, wu, ident], axis=1), bf)


def kernel(x, conv_w, conv_b, attn_w, attn_b, gru_w, gru_u, gru_b):
    x = np.asarray(x, dtype=np.float32)
    conv_w = np.asarray(conv_w, dtype=np.float32)
    conv_b = np.asarray(conv_b, dtype=np.float32)
    attn_w = np.asarray(attn_w, dtype=np.float32)
    attn_b = np.asarray(attn_b, dtype=np.float32)
    gru_w = np.asarray(gru_w, dtype=np.float32)
    gru_u = np.asarray(gru_u, dtype=np.float32)
    gru_b = np.asarray(gru_b, dtype=np.float32)

    zero_bias = (
        not conv_b.any() and not attn_b.any() and not gru_b.any())

    nc = _get_nc(zero_bias)

    xs_bf = x.reshape(B * LTMS, T, C_IN).astype(ml_dtypes.bfloat16)
    bfpack = _pack_weights(conv_w, attn_w, gru_w, gru_u)

    in_maps = []
    for c in range(NCORES):
        m = {
            "x_shard": np.ascontiguousarray(xs_bf[c * S: (c + 1) * S]),
            "bfpack": bfpack,
        }
        if not zero_bias:
            bi, br = gru_b[0], gru_b[1]
            comb = bi + br
            gbr = np.zeros((1, 8 * 128), np.float32)
            gbr[0, 0:512] = comb[0:512]          # z0 z1 r0 r1
            gbr[0, 0:256] *= -1.0                # negated z preact -> w
            gbr[0, 512:768] = br[512:768]        # rh0 rh1
            gbr[0, 768:1024] = bi[512:768]       # xh0 xh1
            m["conv_b2"] = np.ascontiguousarray(
                conv_b.reshape(2, 128).T, np.float32)
            m["attn_b"] = attn_b.reshape(1, T).astype(ml_dtypes.bfloat16)
            m["gbias_row"] = gbr.astype(ml_dtypes.bfloat16)
        in_maps.append(m)

    res = run_bass_kernel_spmd(nc, in_maps, core_ids=list(range(NCORES)))
    outs = [res.results[c]["h_out"] for c in range(NCORES)]
    h = np.concatenate(outs, axis=0)  # [1024, 256]
    return h.reshape(B, LTMS, HH).astype(np.float32)


if __name__ == "__main__":
    nc = _get_nc(True)
    print("built ok")
